# revision 1
# baseline (speedup 1.0000x reference)
"""Distributed GAT kernel for Trainium2 (8 NeuronCores), Bass/Tile.

Architecture (per layer):
  - node tables [TROW*8, 256] fp16 rows = [h(128 f16) | as(4 f32 as 8 f16) | pad],
    replicated to every core via AllGather each layer.
  - each core owns dst shard; edges grouped by (src-group g in 0..3, dst block j
    of 128 local dsts), chunked into 128-edge chunks (count = max over cores).
  - per chunk: dma_gather 512B rows by src; mask one-hot [128e x 128slot] via
    tensor_scalar is_equal; maskT via PE transpose; ad via maskT matmul from
    SBUF ad table; e=lrelu(as+ad); ex=exp(e) -> cols 128:132; weight msgs by ex;
    reduce matmul lhsT=mask rhs=[msgs|ex] accumulated in PSUM per block.
  - block retire: h'acc[:, j, :] += psum. Finalize: normalize by denom cols,
    +bias, ELU, transform with next W, rebuild tables.
  - layer 3: mean heads, +b3, ELU, dot lin_w, pool via Bpool matmul -> [64]
    partial per core; host sums partials (+lin_b).
"""
import numpy as np
import ml_dtypes
from contextlib import ExitStack

import concourse.bacc as bacc
import concourse.bass as bass
import concourse.tile as tile
from concourse import mybir, bass_utils
from concourse.library_config import mlp

F16 = mybir.dt.float16
F32 = mybir.dt.float32
I16 = mybir.dt.int16
NCORES = 8
P = 128
SEGC = 24          # chunks per gather segment
SUP = 8            # chunks per batched-op super-chunk
HEADS = 4
HID = 32
D1 = 128
NEG = 0.2
PAD_AS = -200.0


# ----------------------------------------------------------------------------
# host preprocessing
# ----------------------------------------------------------------------------
def preprocess(edge_index, N):
    """Build the core-independent schedule + per-core index arrays."""
    SH = N // NCORES
    assert SH * NCORES == N
    NBLK = (SH + 127) // 128          # dst blocks per core (last partial)
    TROW = NBLK * 128 + 128           # table rows/core: padded nodes + 1 pad blk
    PADROW = TROW - 1                 # local row used for pad edges
    GSZ = 2 * TROW                    # sub-table rows (2 shards) < 32768
    assert GSZ <= 32768

    E = edge_index.shape[1]
    src = np.concatenate([edge_index[0].astype(np.int64),
                          np.arange(N, dtype=np.int64)])
    dst = np.concatenate([edge_index[1].astype(np.int64),
                          np.arange(N, dtype=np.int64)])

    # per core: sort edges by (group, dst, src); count per (g, block)
    per_core = []
    cnt = np.zeros((NCORES, 4, NBLK), np.int64)
    for c in range(NCORES):
        m = (dst // SH) == c
        s_c, d_c = src[m], dst[m] - c * SH
        # src table row: global row = (src//SH)*TROW + (src%SH); group = row//GSZ
        row = (s_c // SH) * TROW + (s_c % SH)
        g_c = row // GSZ
        order = np.lexsort((s_c, d_c, g_c))
        s_row, d_l, g_c = row[order], d_c[order], g_c[order]
        per_core.append((s_row, d_l, g_c))
        blk = d_l // 128
        np.add.at(cnt[c], (g_c, blk), 1)

    # chunks per (g, block) = cross-core max
    cpb = np.maximum(1, np.ceil(cnt.max(axis=0) / 128).astype(np.int64))  # [4, NBLK]

    # pad each pass's chunk total to a multiple of SEGC (dummy blocks -> block id -1)
    chunk_meta = []   # list of (g, blk, k, first, last) in program order
    for g in range(4):
        tot = 0
        for j in range(NBLK):
            k = int(cpb[g, j])
            for t in range(k):
                chunk_meta.append((g, j, t, t == 0, t == k - 1))
            tot += k
        padn = (-tot) % SEGC
        for t in range(padn):
            chunk_meta.append((g, -1, t, True, True))
    C = len(chunk_meta)

    # per-core edge->chunk-slot assignment
    idx_arrs, d8_arrs = [], []
    for c in range(NCORES):
        s_row, d_l, g_c = per_core[c]
        idx = np.full((C, 128), PADROW, np.int64)   # idx within sub-table
        d8 = np.zeros((C, 128), np.float32)
        # chunk start positions in program order for (g, j)
        start = {}
        pos = 0
        for ci, (g, j, t, fi, la) in enumerate(chunk_meta):
            if j >= 0 and t == 0:
                start[(g, j)] = ci
        for g in range(4):
            mg = g_c == g
            sg, dg = s_row[mg], d_l[mg]
            blocks = dg // 128
            for j in np.unique(blocks):
                mb = blocks == j
                rows, dl = sg[mb], dg[mb]
                c0 = start[(g, int(j))]
                n = len(rows)
                ch = np.arange(n) // 128
                sl = np.arange(n) % 128
                idx[c0 + ch, sl] = rows - g * GSZ
                d8[c0 + ch, sl] = dl - int(j) * 128
        # dma_gather layout: idx i -> partition i%16, col i//16; replicate x8
        flat = idx.reshape(-1)
        il = np.zeros((16, C * 8), np.int16)
        ar = np.arange(C * 128)
        il[ar % 16, ar // 16] = flat.astype(np.int16)
        idx_arrs.append(np.tile(il, (8, 1)))
        d8_arrs.append(np.ascontiguousarray(d8.reshape(C, 128).T))  # [128, C]
    sched = dict(SH=SH, NBLK=NBLK, TROW=TROW, PADROW=PADROW, GSZ=GSZ, C=C,
                 chunk_meta=chunk_meta)
    return sched, idx_arrs, d8_arrs


# ----------------------------------------------------------------------------
# device program
# ----------------------------------------------------------------------------
def build_program(sched, repeat=1):
    SH, NBLK, TROW, C = sched["SH"], sched["NBLK"], sched["TROW"], sched["C"]
    GSZ = sched["GSZ"]
    chunk_meta = sched["chunk_meta"]
    NROW = NBLK * 128

    nc = bacc.Bacc("TRN2", target_bir_lowering=False, debug=False,
                   num_devices=NCORES)

    xT = nc.dram_tensor("xT", [P, TROW], F32, kind="ExternalInput")
    idx16 = nc.dram_tensor("idx16", [P, C * 8], I16, kind="ExternalInput")
    d8col = nc.dram_tensor("d8col", [P, C], F32, kind="ExternalInput")
    WT = [nc.dram_tensor(f"WT{l}", [P, P], F32, kind="ExternalInput")
          for l in range(3)]
    asrep = [nc.dram_tensor(f"asrep{l}", [P, P], F16, kind="ExternalInput")
             for l in range(3)]
    adrep = [nc.dram_tensor(f"adrep{l}", [P, P], F16, kind="ExternalInput")
             for l in range(3)]
    brep = [nc.dram_tensor(f"brep{l}", [P, P], F32, kind="ExternalInput")
            for l in range(2)]
    b3rep = nc.dram_tensor("b3rep", [P, HID], F32, kind="ExternalInput")
    lwrep = nc.dram_tensor("lwrep", [P, HID], F32, kind="ExternalInput")
    iotarep = nc.dram_tensor("iotarep", [P, P], F16, kind="ExternalInput")
    ident = nc.dram_tensor("ident", [P, P], F16, kind="ExternalInput")
    ident32 = nc.dram_tensor("ident32", [P, P], F32, kind="ExternalInput")
    padblk = nc.dram_tensor("padblk", [P, 256], F16, kind="ExternalInput")
    bpool = nc.dram_tensor("bpool", [P, NBLK * 64], F32, kind="ExternalInput")
    out64 = nc.dram_tensor("out64", [64, 1], F32, kind="ExternalOutput")

    with tile.TileContext(nc) as tc, ExitStack() as ctx:
        sb = ctx.enter_context(tc.tile_pool(name="sb", bufs=2))
        sbc = ctx.enter_context(tc.tile_pool(name="sbc", bufs=1))
        ps = ctx.enter_context(tc.tile_pool(name="ps", bufs=2, space="PSUM"))
        dr = ctx.enter_context(tc.tile_pool(name="dr", bufs=1, space="DRAM"))

        nc.gpsimd.load_library(mlp)

        iota_sb = sbc.tile([P, P], F16)
        nc.sync.dma_start(iota_sb[:], iotarep[:])
        id_sb = sbc.tile([P, P], F16)
        nc.sync.dma_start(id_sb[:], ident[:])
        id32_sb = sbc.tile([P, P], F32)
        nc.sync.dma_start(id32_sb[:], ident32[:])
        acc = sbc.tile([P, NBLK, 132], F32)
        ad_sb = sbc.tile([P, NBLK, 4], F16)
        hnode = sbc.tile([P, NBLK, P], F16)
        hT = sbc.tile([P, TROW], F16)
        yv = acc[:, :, 0:128]

        tbl_in = dr.tile([TROW, 256], F16)
        tbl_full = dr.tile([TROW * NCORES, 256], F16)

        wt_sb = [sbc.tile([P, P], F32, tag=f"wt{l}", name=f"wt{l}") for l in range(3)]
        as_sb = [sbc.tile([P, P], F16, tag=f"asw{l}", name=f"asw{l}") for l in range(3)]
        adw_sb = [sbc.tile([P, P], F16, tag=f"adw{l}", name=f"adw{l}") for l in range(3)]
        b_sb = [sbc.tile([P, P], F32, tag=f"bb{l}", name=f"bb{l}") for l in range(2)]
        b3_sb = sbc.tile([P, HID], F32)
        lw_sb = sbc.tile([P, HID], F32)
        for l in range(3):
            nc.sync.dma_start(wt_sb[l][:], WT[l][:])
            nc.sync.dma_start(as_sb[l][:], asrep[l][:])
            nc.sync.dma_start(adw_sb[l][:], adrep[l][:])
        for l in range(2):
            nc.sync.dma_start(b_sb[l][:], brep[l][:])
        nc.sync.dma_start(b3_sb[:], b3rep[:])
        nc.sync.dma_start(lw_sb[:], lwrep[:])

        def w_transform_x():
            pos = 0
            while pos < TROW:
                w = min(512, TROW - pos)
                xs = sb.tile([P, 512], F32, tag="xs")
                nc.sync.dma_start(xs[:, :w], xT[:, pos:pos + w])
                hps = ps.tile([P, 512], F32, tag="big", space="PSUM")
                nc.tensor.matmul(hps[:, :w], lhsT=wt_sb[0][:],
                                 rhs=xs[:, :w], start=True, stop=True)
                nc.scalar.copy(hT[:, pos:pos + w], hps[:, :w])
                pos += w

        def w_transform_y(l):
            # fused: transpose yv (4 blocks) -> yT chunk -> W matmul -> hT
            for q in range((NBLK + 3) // 4):
                s0 = q * 4
                nb = min(4, NBLK - s0)
                ytc = sb.tile([P, 512], F32, tag="ytc")
                for t in range(nb):
                    tp = ps.tile([P, P], F32, tag="big", space="PSUM")
                    nc.tensor.transpose(out=tp[:], in_=yv[:, s0 + t, :],
                                        identity=id32_sb[:])
                    nc.scalar.copy(ytc[:, t * P:(t + 1) * P], tp[:])
                hps = ps.tile([P, 512], F32, tag="big", space="PSUM")
                nc.tensor.matmul(hps[:, :nb * P], lhsT=wt_sb[l][:],
                                 rhs=ytc[:, :nb * P], start=True, stop=True)
                nc.scalar.copy(hT[:, s0 * P:s0 * P + nb * P], hps[:, :nb * P])
            if NROW < TROW:
                nc.vector.memset(hT[:, NROW:TROW], 0.0)

        def build_tables(l):
            nc.sync.dma_start_transpose(hnode[:], hT[:, 0:NROW])
            as_node = sb.tile([P, NBLK, 4], F32, tag="asred")
            ad_node = sb.tile([P, NBLK, 4], F32, tag="adred")
            QB = 13
            for rep, red in ((as_sb[l], as_node), (adw_sb[l], ad_node)):
                for q0 in range(0, NBLK, QB):
                    qn = min(QB, NBLK - q0)
                    tmp = sb.tile([P, QB, P], F16, tag="ashtmp")
                    nc.vector.tensor_tensor(
                        out=tmp[:, :qn, :], in0=hnode[:, q0:q0 + qn, :],
                        in1=rep[:].unsqueeze(1).to_broadcast([P, qn, P]),
                        op=mybir.AluOpType.mult)
                    nc.vector.tensor_reduce(
                        out=red[:, q0:q0 + qn, :],
                        in_=tmp[:, :qn, :].rearrange("p s (h c) -> p s h c", h=4),
                        axis=mybir.AxisListType.X, op=mybir.AluOpType.add)
            nc.vector.tensor_copy(ad_sb[:], ad_node[:])
            nc.sync.dma_start(
                tbl_in[0:NROW, 0:P].rearrange("(s p) f -> p s f", p=P), hnode[:])
            nc.sync.dma_start(
                tbl_in[:].bitcast(F32)[0:NROW, 64:68].rearrange(
                    "(s p) f -> p s f", p=P), as_node[:])
            nc.sync.dma_start(
                tbl_in[NROW:TROW, :].rearrange("(s p) f -> p s f", p=P),
                padblk[:].unsqueeze(1))
            nc.gpsimd.collective_compute(
                "AllGather", mybir.AluOpType.bypass,
                replica_groups=[list(range(NCORES))],
                ins=[tbl_in[:].opt()], outs=[tbl_full[:].opt()])

        def run_edges():
            nc.vector.memset(acc[:, :, 0:128], 0.0)
            nc.vector.memset(acc[:, :, 128:132], 1e-30)
            grp_ps = {}
            nseg = C // SEGC
            for seg in range(nseg):
                c0 = seg * SEGC
                g = chunk_meta[c0][0]
                msgs = sb.tile([P, SEGC, 256], F16, tag="msgs")
                idx_sb = sb.tile([P, SEGC * 8], I16, tag="idxseg")
                nc.sync.dma_start(idx_sb[:], idx16[:, c0 * 8:(c0 + SEGC) * 8])
                d8_sb = sb.tile([P, SEGC], F32, tag="d8seg")
                nc.sync.dma_start(d8_sb[:], d8col[:, c0:c0 + SEGC])
                nidx = SEGC * 128
                nc.gpsimd.dma_gather(
                    msgs[:], tbl_full[g * GSZ:(g + 1) * GSZ, :],
                    idx_sb[:], nidx, nidx, 256,
                    single_packet=False)
                for su in range(SEGC // SUP):
                    k0 = su * SUP
                    maskS = sb.tile([P, SUP, P], F16, tag="maskS")
                    ad_ps = ps.tile([P, SUP, 4], F32, tag="adps", space="PSUM")
                    for k in range(SUP):
                        ci = c0 + k0 + k
                        _, j, t, first, last = chunk_meta[ci]
                        jj = max(j, 0)
                        nc.vector.tensor_scalar(
                            out=maskS[:, k, :], in0=iota_sb[:],
                            scalar1=d8_sb[:, k0 + k:k0 + k + 1], scalar2=None,
                            op0=mybir.AluOpType.is_equal)
                        mtp = ps.tile([P, P], F16, tag="mtp", space="PSUM")
                        nc.tensor.transpose(out=mtp[:], in_=maskS[:, k, :],
                                            identity=id_sb[:])
                        mts = sb.tile([P, P], F16, tag="mts")
                        nc.scalar.copy(mts[:], mtp[:])
                        nc.tensor.matmul(ad_ps[:, k, :], lhsT=mts[:],
                                         rhs=ad_sb[:, jj, :], start=True,
                                         stop=True)
                    sl = slice(k0, k0 + SUP)
                    asv = msgs[:].bitcast(F32)[:, sl, 64:68]
                    e1 = sb.tile([P, SUP, 4], F32, tag="e1")
                    nc.vector.tensor_tensor(out=e1[:], in0=asv, in1=ad_ps[:],
                                            op=mybir.AluOpType.add)
                    e2 = sb.tile([P, SUP, 4], F32, tag="e2")
                    nc.vector.scalar_tensor_tensor(
                        out=e2[:], in0=e1[:], scalar=NEG, in1=e1[:],
                        op0=mybir.AluOpType.mult, op1=mybir.AluOpType.max)
                    nc.scalar.activation(msgs[:, sl, 128:132], e2[:],
                                         mybir.ActivationFunctionType.Exp)
                    nc.vector.tensor_tensor(
                        out=msgs[:, sl, 0:128].rearrange(
                            "p s (h c) -> p s h c", h=4),
                        in0=msgs[:, sl, 0:128].rearrange(
                            "p s (h c) -> p s h c", h=4),
                        in1=msgs[:, sl, 128:132].unsqueeze(3).to_broadcast(
                            [P, SUP, 4, 32]),
                        op=mybir.AluOpType.mult)
                    for k in range(SUP):
                        ci = c0 + k0 + k
                        g2, j, t, first, last = chunk_meta[ci]
                        key = (g2, j) if j >= 0 else ("dummy", ci)
                        if first:
                            grp_ps[key] = ps.tile([P, 132], F32, tag="grp",
                                                  name="grp", space="PSUM")
                        gp = grp_ps[key]
                        nc.tensor.matmul(gp[:], lhsT=maskS[:, k, :],
                                         rhs=msgs[:, k0 + k, 0:132],
                                         start=first, stop=last)
                        if last and j >= 0:
                            nc.vector.tensor_tensor(
                                out=acc[:, j, :], in0=acc[:, j, :], in1=gp[:],
                                op=mybir.AluOpType.add)

        def elu_inplace(full_ap, nblk, width):
            EB = 4
            for q0 in range(0, nblk, EB):
                qn = min(EB, nblk - q0)
                ap = full_ap[:, q0:q0 + qn, :]
                shape = [P, EB, width]
                a = sb.tile(shape, F32, tag="elua")
                nc.scalar.activation(a[:, :qn, :], ap,
                                     mybir.ActivationFunctionType.Relu)
                bmin = sb.tile(shape, F32, tag="elub")
                nc.vector.tensor_scalar(out=bmin[:, :qn, :], in0=ap, scalar1=0.0,
                                        scalar2=None, op0=mybir.AluOpType.min)
                cc = sb.tile(shape, F32, tag="eluc")
                nc.scalar.activation(cc[:, :qn, :], bmin[:, :qn, :],
                                     mybir.ActivationFunctionType.Exp)
                nc.vector.scalar_tensor_tensor(
                    out=ap, in0=a[:, :qn, :], scalar=-1.0, in1=cc[:, :qn, :],
                    op0=mybir.AluOpType.add, op1=mybir.AluOpType.add)

        def finalize(l):
            rec = sb.tile([P, NBLK, 4], F32, tag="rec")
            nc.vector.reciprocal(out=rec[:], in_=acc[:, :, 128:132])
            nc.vector.tensor_tensor(
                out=yv[:].rearrange("p s (h c) -> p s h c", h=4),
                in0=acc[:, :, 0:128].rearrange("p s (h c) -> p s h c", h=4),
                in1=rec[:].unsqueeze(3).to_broadcast([P, NBLK, 4, 32]),
                op=mybir.AluOpType.mult)
            if l < 2:
                nc.vector.tensor_tensor(
                    out=yv[:], in0=yv[:],
                    in1=b_sb[l][:].unsqueeze(1).to_broadcast([P, NBLK, P]),
                    op=mybir.AluOpType.add)
                elu_inplace(yv, NBLK, P)


        for _rep in range(repeat):
          for l in range(3):
            if l == 0:
                w_transform_x()
            else:
                w_transform_y(l)
            build_tables(l)
            run_edges()
            finalize(l)

        h3 = sb.tile([P, NBLK, HID], F32, tag="h3")
        nc.vector.tensor_reduce(
            out=h3[:], in_=yv[:].rearrange("p s (h c) -> p s c h", h=4),
            axis=mybir.AxisListType.X, op=mybir.AluOpType.add)
        nc.vector.tensor_scalar(out=h3[:], in0=h3[:], scalar1=0.25,
                                scalar2=None, op0=mybir.AluOpType.mult)
        nc.vector.tensor_tensor(
            out=h3[:], in0=h3[:],
            in1=b3_sb[:].unsqueeze(1).to_broadcast([P, NBLK, HID]),
            op=mybir.AluOpType.add)
        elu_inplace(h3, NBLK, HID)
        pv = sb.tile([P, NBLK], F32, tag="pv")
        for q0 in range(0, NBLK, 16):
            qn = min(16, NBLK - q0)
            tmp3 = sb.tile([P, 16, HID], F32, tag="tmp3")
            nc.vector.tensor_tensor(
                out=tmp3[:, :qn, :], in0=h3[:, q0:q0 + qn, :],
                in1=lw_sb[:].unsqueeze(1).to_broadcast([P, qn, HID]),
                op=mybir.AluOpType.mult)
            nc.vector.tensor_reduce(out=pv[:, q0:q0 + qn], in_=tmp3[:, :qn, :],
                                    axis=mybir.AxisListType.X,
                                    op=mybir.AluOpType.add)
        pool_ps = ps.tile([64, 1], F32, tag="big", space="PSUM")
        for s in range(NBLK):
            bps = sb.tile([P, 64], F32, tag="bps")
            nc.sync.dma_start(bps[:], bpool[:, s * 64:(s + 1) * 64])
            nc.tensor.matmul(pool_ps[:], lhsT=bps[:], rhs=pv[:, s:s + 1],
                             start=(s == 0), stop=(s == NBLK - 1))
        pool_sb = sb.tile([64, 1], F32, tag="poolsb")
        nc.scalar.copy(pool_sb[:], pool_ps[:])
        nc.sync.dma_start(out64[:], pool_sb[:])

    nc.compile()
    return nc


# ----------------------------------------------------------------------------
# host-side input construction
# ----------------------------------------------------------------------------
def make_inputs(sched, idx_arrs, d8_arrs, inputs, batch_counts=None):
    """Per-core in_maps from the raw problem inputs dict."""
    SH, NBLK, TROW = sched["SH"], sched["NBLK"], sched["TROW"]
    NROW = NBLK * 128
    x = np.asarray(inputs["x"], np.float32)
    N = x.shape[0]
    batch = np.asarray(inputs["batch"], np.int64)
    NGr = 64 if batch_counts is None else len(batch_counts)
    counts = np.bincount(batch, minlength=NGr).astype(np.float32)
    counts[counts == 0] = 1.0

    def rep(v, width=P):
        v = np.asarray(v, np.float32).reshape(1, -1)
        return np.tile(v, (P, 1))

    Ws = [np.asarray(inputs[k], np.float32).T.copy() for k in ("W1", "W2", "W3")]
    asr = [rep(np.asarray(inputs[k], np.float32).reshape(-1)).astype(np.float16)
           for k in ("a1s", "a2s", "a3s")]
    adr = [rep(np.asarray(inputs[k], np.float32).reshape(-1)).astype(np.float16)
           for k in ("a1d", "a2d", "a3d")]
    br = [rep(inputs["b1"]), rep(inputs["b2"])]
    b3r = rep(inputs["b3"])
    lwr = rep(np.asarray(inputs["lin_w"], np.float32).reshape(-1))
    iot = np.tile(np.arange(P, dtype=np.float32), (P, 1)).astype(np.float16)
    idf = np.eye(P, dtype=np.float16)
    idf32 = np.eye(P, dtype=np.float32)
    pad = np.zeros((P, 256), np.float16)
    pad.view(np.float32)[:, 64:68] = PAD_AS

    in_maps = []
    for c in range(NCORES):
        xs = np.zeros((TROW, P), np.float32)
        xs[0:SH] = x[c * SH:(c + 1) * SH]
        bp = np.zeros((NROW, 64), np.float32)
        b_loc = batch[c * SH:(c + 1) * SH]
        bp[np.arange(SH), b_loc] = 1.0 / counts[b_loc]
        m = {"xT": np.ascontiguousarray(xs.T), "idx16": idx_arrs[c],
             "d8col": d8_arrs[c],
             "b3rep": b3r, "lwrep": lwr, "iotarep": iot, "ident": idf,
             "ident32": idf32, "padblk": pad,
             "bpool": np.ascontiguousarray(
                 bp.reshape(NBLK, P, 64).transpose(1, 0, 2).reshape(P, NBLK * 64))}
        for l in range(3):
            m[f"WT{l}"] = Ws[l]
            m[f"asrep{l}"] = asr[l]
            m[f"adrep{l}"] = adr[l]
        for l in range(2):
            m[f"brep{l}"] = br[l]
        in_maps.append(m)
    return in_maps


# ----------------------------------------------------------------------------
# SPMD runner (modeled on bass2jax.run_bass_via_pjrt, with reusable executable)
# ----------------------------------------------------------------------------
def make_runner(nc, in_maps):
    import jax
    import jax.numpy as jnp
    from jax.sharding import Mesh, PartitionSpec
    from jax.experimental.shard_map import shard_map
    from concourse import bass2jax, mybir as mb

    bass2jax.install_neuronx_cc_hook()
    n_cores = len(in_maps)
    part_name = nc.partition_id_tensor.name if nc.partition_id_tensor else None
    in_names, out_names, out_avals, zero_outs = [], [], [], []
    for alloc in nc.m.functions[0].allocations:
        if not isinstance(alloc, mb.MemoryLocationSet):
            continue
        name = alloc.memorylocations[0].name
        if alloc.kind == "ExternalInput":
            if name != part_name:
                in_names.append(name)
        elif alloc.kind == "ExternalOutput":
            out_names.append(name)
            shape = tuple(alloc.tensor_shape)
            dtype = mb.dt.np(alloc.dtype)
            out_avals.append(jax.core.ShapedArray(shape, dtype))
            zero_outs.append(np.zeros(shape, dtype))
    n_params = len(in_names)
    all_names = in_names + out_names
    if part_name is not None:
        all_names = all_names + [part_name]

    def _body(*args):
        operands = list(args)
        if part_name is not None:
            operands.append(bass2jax.partition_id_tensor())
        outs = bass2jax._bass_exec_p.bind(
            *operands, out_avals=tuple(out_avals), in_names=tuple(all_names),
            out_names=tuple(out_names), lowering_input_output_aliases=(),
            sim_require_finite=False, sim_require_nnan=False, nc=nc)
        return tuple(outs)

    devices = jax.devices()[:n_cores]
    mesh = Mesh(np.asarray(devices), ("core",))
    in_specs = (PartitionSpec("core"),) * (n_params + len(out_names))
    out_specs = (PartitionSpec("core"),) * len(out_names)
    fn = jax.jit(shard_map(_body, mesh=mesh, in_specs=in_specs,
                           out_specs=out_specs, check_rep=False))
    concat_in = [np.concatenate([np.asarray(in_maps[c][nm])
                                 for c in range(n_cores)], axis=0)
                 for nm in in_names]
    concat_zeros = [np.zeros((n_cores * z.shape[0], *z.shape[1:]), z.dtype)
                    for z in zero_outs]
    dev_in = [jax.device_put(
        a, jax.sharding.NamedSharding(mesh, PartitionSpec("core")))
        for a in concat_in + concat_zeros]

    def run():
        outs = fn(*dev_in)
        outs = [np.asarray(o) for o in outs]
        return [
            {nm: outs[i].reshape(n_cores, *out_avals[i].shape)[c]
             for i, nm in enumerate(out_names)}
            for c in range(n_cores)]
    return run


def kernel(**inputs):
    """Full-input distributed GAT kernel; returns pooled [64] float32."""
    inputs = {k: np.asarray(v) for k, v in inputs.items()}
    N = inputs["x"].shape[0]
    sched, idx_arrs, d8_arrs = preprocess(inputs["edge_index"], N)
    nc = build_program(sched)
    in_maps = make_inputs(sched, idx_arrs, d8_arrs, inputs)
    run = make_runner(nc, in_maps)
    kernel.last_runner = run          # exposed for test.py timing
    kernel.last_inputs = inputs
    results = run()
    partial = sum(r["out64"][:, 0] for r in results)
    out = (partial + np.float32(inputs["lin_b"].reshape(-1)[0]))[:64]
    return out.astype(np.float32)



# revision 11
# speedup vs baseline: 1.9597x; 1.9597x over previous
"""Distributed GAT kernel for Trainium2 (8 NeuronCores), Bass/Tile. v2

Architecture (per layer):
  - node tables [TROW*8, 256B] rows = [h (128 f8e4) | as (4 f16) | pad],
    built per-core then replicated via 4 quarter AllGathers (overlap with
    edge compute of earlier quarters).
  - each core owns a dst shard; edges grouped by (src quarter g in 0..3,
    dst block j of 128 local dsts), chunked into 128-edge chunks (count =
    max over cores, so the SPMD program is identical on every core).
  - per segment (24 chunks): 4-queue dma_gather of 256B rows by src;
    ONE batched is_equal builds all 24 one-hot dst masks; per 8-chunk
    sub-batch: PE transposes masks into one PSUM bank, ONE Act copy out,
    per-chunk 4-col matmul gathers ad[dst]; attention e=exp(lrelu(as+ad))
    batched; weighted messages [a*h | e] in one 132-col tile; per-chunk
    scatter matmul accumulates [msgs|denom] per dst block in PSUM.
  - finalize: normalize by denom, +bias, ELU, transform with next W.
  - layer 3: mean heads, +b3, ELU, dot lin_w, pool via Bpool matmul ->
    [64] partial per core; host sums partials (+lin_b).
"""
import numpy as np
import ml_dtypes
from contextlib import ExitStack

import concourse.bacc as bacc
import concourse.bass as bass
import concourse.tile as tile
from concourse import mybir, bass_utils
from concourse.library_config import mlp

F16 = mybir.dt.float16
F32 = mybir.dt.float32
F8 = mybir.dt.float8e4
U8 = mybir.dt.uint8
I16 = mybir.dt.int16
NCORES = 8
P = 128
SEGC = 24          # chunks per gather segment
SUB = 8            # chunks per batched-op sub-batch
HEADS = 4
HID = 32
D1 = 128
NEG = 0.2
ROWB = 256         # table row bytes


# ----------------------------------------------------------------------------
# host preprocessing
# ----------------------------------------------------------------------------
def preprocess(edge_index, N):
    """Build the core-independent schedule + per-core index arrays."""
    SH = N // NCORES
    assert SH * NCORES == N
    NBLK = (SH + 127) // 128          # dst blocks per core (last partial)
    TROW = NBLK * 128 + 128           # table rows/core: padded nodes + pad blk
    QROW = TROW // 4                  # rows per quarter (group)
    GSZ = NCORES * QROW               # rows per group table
    assert GSZ <= 32768

    E = edge_index.shape[1]
    src = np.concatenate([edge_index[0].astype(np.int64),
                          np.arange(N, dtype=np.int64)])
    dst = np.concatenate([edge_index[1].astype(np.int64),
                          np.arange(N, dtype=np.int64)])

    # per core: sort edges by (group, dst, srcrow); count per (g, block)
    per_core = []
    cnt = np.zeros((NCORES, 4, NBLK), np.int64)
    for c in range(NCORES):
        m = (dst // SH) == c
        s_c, d_c = src[m], dst[m] - c * SH
        sc, l = s_c // SH, s_c % SH
        g_c = np.minimum(l // QROW, 3)
        grow = sc * QROW + (l - g_c * QROW)
        order = np.lexsort((grow, d_c, g_c))
        s_row, d_l, g_c = grow[order], d_c[order], g_c[order]
        per_core.append((s_row, d_l, g_c))
        blk = d_l // 128
        np.add.at(cnt[c], (g_c, blk), 1)

    # chunks per (g, block) = cross-core max
    cpb = np.maximum(1, np.ceil(cnt.max(axis=0) / 128).astype(np.int64))

    # pad each group's chunk total to a multiple of SEGC (dummy block id -1)
    chunk_meta = []   # list of (g, blk, k, first, last) in program order
    for g in range(4):
        tot = 0
        for j in range(NBLK):
            k = int(cpb[g, j])
            for t in range(k):
                chunk_meta.append((g, j, t, t == 0, t == k - 1))
            tot += k
        padn = (-tot) % SEGC
        for t in range(padn):
            chunk_meta.append((g, -1, t, True, True))
    C = len(chunk_meta)

    # per-core edge->chunk-slot assignment
    idx_arrs, d8_arrs = [], []
    for c in range(NCORES):
        s_row, d_l, g_c = per_core[c]
        idx = np.zeros((C, 128), np.int64)          # idx within group table
        d8 = np.full((C, 128), 128.0, np.float32)   # pad slots kill the mask
        start = {}
        for ci, (g, j, t, fi, la) in enumerate(chunk_meta):
            if j >= 0 and t == 0:
                start[(g, j)] = ci
        for g in range(4):
            mg = g_c == g
            sg, dg = s_row[mg], d_l[mg]
            blocks = dg // 128
            for j in np.unique(blocks):
                mb = blocks == j
                rows, dl = sg[mb], dg[mb]
                c0 = start[(g, int(j))]
                n = len(rows)
                ch = np.arange(n) // 128
                sl = np.arange(n) % 128
                idx[c0 + ch, sl] = rows
                d8[c0 + ch, sl] = dl - int(j) * 128
        # dma_gather layout: idx i -> partition i%16, col i//16; replicate x8
        flat = idx.reshape(-1)
        il = np.zeros((16, C * 8), np.int16)
        ar = np.arange(C * 128)
        il[ar % 16, ar // 16] = flat.astype(np.int16)
        idx_arrs.append(np.tile(il, (8, 1)))
        d8_arrs.append(np.ascontiguousarray(
            d8.reshape(C, 128).T).astype(np.float16))  # [128, C]
    sched = dict(SH=SH, NBLK=NBLK, TROW=TROW, QROW=QROW, GSZ=GSZ, C=C,
                 chunk_meta=chunk_meta)
    return sched, idx_arrs, d8_arrs


# ----------------------------------------------------------------------------
# device program
# ----------------------------------------------------------------------------
def build_program(sched, repeat=1, nq=4):
    SH, NBLK, TROW, C = sched["SH"], sched["NBLK"], sched["TROW"], sched["C"]
    QROW, GSZ = sched["QROW"], sched["GSZ"]
    chunk_meta = sched["chunk_meta"]
    NROW = NBLK * 128

    nc = bacc.Bacc("TRN2", target_bir_lowering=False, debug=False,
                   num_devices=NCORES, num_swdge_queues=max(nq, 1))

    xT = nc.dram_tensor("xT", [P, TROW], F32, kind="ExternalInput")
    idx16 = nc.dram_tensor("idx16", [P, C * 8], I16, kind="ExternalInput")
    d8col = nc.dram_tensor("d8col", [P, C], F16, kind="ExternalInput")
    WT = [nc.dram_tensor(f"WT{l}", [P, P], F32, kind="ExternalInput")
          for l in range(3)]
    asrep = [nc.dram_tensor(f"asrep{l}", [P, P], F16, kind="ExternalInput")
             for l in range(3)]
    adrep = [nc.dram_tensor(f"adrep{l}", [P, P], F16, kind="ExternalInput")
             for l in range(3)]
    brep = [nc.dram_tensor(f"brep{l}", [P, P], F32, kind="ExternalInput")
            for l in range(2)]
    b3rep = nc.dram_tensor("b3rep", [P, HID], F32, kind="ExternalInput")
    lwrep = nc.dram_tensor("lwrep", [P, HID], F32, kind="ExternalInput")
    iotarep = nc.dram_tensor("iotarep", [P, P], F16, kind="ExternalInput")
    ident = nc.dram_tensor("ident", [P, P], F16, kind="ExternalInput")
    ident32 = nc.dram_tensor("ident32", [P, P], F32, kind="ExternalInput")
    bpool = nc.dram_tensor("bpool", [P, NBLK * 64], F32, kind="ExternalInput")
    out64 = nc.dram_tensor("out64", [64, 1], F32, kind="ExternalOutput")

    with tile.TileContext(nc) as tc, ExitStack() as ctx:
        sb = ctx.enter_context(tc.tile_pool(name="sb", bufs=2))
        sbc = ctx.enter_context(tc.tile_pool(name="sbc", bufs=1))
        ps = ctx.enter_context(tc.tile_pool(name="ps", bufs=2, space="PSUM"))
        dr = ctx.enter_context(tc.tile_pool(name="dr", bufs=1, space="DRAM"))

        nc.gpsimd.load_library(mlp)

        iota_sb = sbc.tile([P, P], F16)
        nc.sync.dma_start(iota_sb[:], iotarep[:])
        id_sb = sbc.tile([P, P], F16)
        nc.sync.dma_start(id_sb[:], ident[:])
        id32_sb = sbc.tile([P, P], F32)
        nc.sync.dma_start(id32_sb[:], ident32[:])
        acc = sbc.tile([P, NBLK, 132], F32)
        ad_sb = sbc.tile([P, NBLK, 4], F16)
        as_node = sbc.tile([P, NBLK, 4], F16)
        hT = sbc.tile([P, TROW], F16)
        yv = acc[:, :, 0:128]

        tbl_in = dr.tile([TROW, ROWB], U8)
        tbl_grp = [None] * 4

        wt_sb = [sbc.tile([P, P], F32, tag=f"wt{l}", name=f"wt{l}") for l in range(3)]
        as_sb = [sbc.tile([P, P], F16, tag=f"asw{l}", name=f"asw{l}") for l in range(3)]
        adw_sb = [sbc.tile([P, P], F16, tag=f"adw{l}", name=f"adw{l}") for l in range(3)]
        b_sb = [sbc.tile([P, P], F32, tag=f"bb{l}", name=f"bb{l}") for l in range(2)]
        b3_sb = sbc.tile([P, HID], F32)
        lw_sb = sbc.tile([P, HID], F32)
        for l in range(3):
            nc.sync.dma_start(wt_sb[l][:], WT[l][:])
            nc.sync.dma_start(as_sb[l][:], asrep[l][:])
            nc.sync.dma_start(adw_sb[l][:], adrep[l][:])
        for l in range(2):
            nc.sync.dma_start(b_sb[l][:], brep[l][:])
        nc.sync.dma_start(b3_sb[:], b3rep[:])
        nc.sync.dma_start(lw_sb[:], lwrep[:])

        # zero the table pad rows once (content: h=0, as=0 -> harmless)
        zrow = sbc.tile([P, ROWB], U8)
        nc.vector.memset(zrow[:], 0.0)
        nc.sync.dma_start(
            tbl_in[NROW:TROW, :].rearrange("(s p) f -> p s f", p=P),
            zrow[:].unsqueeze(1))

        def w_transform_x():
            pos = 0
            while pos < TROW:
                w = min(512, TROW - pos)
                xs = sb.tile([P, 512], F32, tag="xs")
                nc.sync.dma_start(xs[:, :w], xT[:, pos:pos + w])
                hps = ps.tile([P, 512], F32, tag="big", space="PSUM")
                nc.tensor.matmul(hps[:, :w], lhsT=wt_sb[0][:],
                                 rhs=xs[:, :w], start=True, stop=True)
                nc.scalar.copy(hT[:, pos:pos + w], hps[:, :w])
                pos += w

        def w_transform_y(l):
            # fused: transpose yv (4 blocks) -> yT chunk -> W matmul -> hT
            for q in range((NBLK + 3) // 4):
                s0 = q * 4
                nb = min(4, NBLK - s0)
                ytc = sb.tile([P, 512], F32, tag="ytc")
                for t in range(nb):
                    tp = ps.tile([P, P], F32, tag="big", space="PSUM")
                    nc.tensor.transpose(out=tp[:], in_=yv[:, s0 + t, :],
                                        identity=id32_sb[:])
                    nc.scalar.copy(ytc[:, t * P:(t + 1) * P], tp[:])
                hps = ps.tile([P, 512], F32, tag="big", space="PSUM")
                nc.tensor.matmul(hps[:, :nb * P], lhsT=wt_sb[l][:],
                                 rhs=ytc[:, :nb * P], start=True, stop=True)
                nc.scalar.copy(hT[:, s0 * P:s0 * P + nb * P], hps[:, :nb * P])
            if NROW < TROW:
                nc.vector.memset(hT[:, NROW:TROW], 0.0)

        def build_tables(l):
            QB = 25
            for q0 in range(0, NBLK, QB):
                qn = min(QB, NBLK - q0)
                hb = sb.tile([P, QB, P], F16, tag="hnodeb")
                nc.sync.dma_start_transpose(
                    hb[:, :qn, :], hT[:, q0 * P:(q0 + qn) * P])
                for rep, red in ((as_sb[l], as_node), (adw_sb[l], ad_sb)):
                    tmp = sb.tile([P, QB, P], F16, tag="ashtmp")
                    nc.vector.tensor_tensor(
                        out=tmp[:, :qn, :], in0=hb[:, :qn, :],
                        in1=rep[:].unsqueeze(1).to_broadcast([P, qn, P]),
                        op=mybir.AluOpType.mult)
                    with nc.allow_low_precision(reason="32-term sums; f16 ok"):
                        nc.vector.tensor_reduce(
                            out=red[:, q0:q0 + qn, :],
                            in_=tmp[:, :qn, :].rearrange(
                                "p s (h c) -> p s h c", h=4),
                            axis=mybir.AxisListType.X, op=mybir.AluOpType.add)
                hb8 = sb.tile([P, QB, P], F8, tag="hnodeb8")
                nc.scalar.copy(hb8[:, :qn, :], hb[:, :qn, :])
                nc.sync.dma_start(
                    tbl_in[q0 * P:(q0 + qn) * P, 0:P].rearrange(
                        "(s p) f -> p s f", p=P),
                    hb8[:, :qn, :].bitcast(U8))
                nc.sync.dma_start(
                    tbl_in[:].bitcast(F16)[q0 * P:(q0 + qn) * P, 64:68].rearrange(
                        "(s p) f -> p s f", p=P), as_node[:, q0:q0 + qn, :])
            for q in range(4):
                tbl_grp[q] = dr.tile([GSZ, ROWB], U8, name=f"tblg{l}_{q}",
                                     addr_space="Shared")
                nc.gpsimd.collective_compute(
                    "AllGather", mybir.AluOpType.bypass,
                    replica_groups=[list(range(NCORES))],
                    ins=[tbl_in[q * QROW:(q + 1) * QROW, :].opt()],
                    outs=[tbl_grp[q][:].opt()])

        def run_edges():
            nc.vector.memset(acc[:, :, 0:128], 0.0)
            nc.vector.memset(acc[:, :, 128:132], 1e-30)
            grp_ps = {}
            nseg = C // SEGC
            for seg in range(nseg):
                c0 = seg * SEGC
                g = chunk_meta[c0][0]
                msgs = sb.tile([P, SEGC, ROWB], U8, tag="msgs")
                idx_sb = sb.tile([P, SEGC * 8], I16, tag="idxseg")
                nc.sync.dma_start(idx_sb[:], idx16[:, c0 * 8:(c0 + SEGC) * 8])
                d8_sb = sb.tile([P, SEGC], F16, tag="d8seg")
                nc.sync.dma_start(d8_sb[:], d8col[:, c0:c0 + SEGC])
                qc = SEGC // nq
                for q in range(nq):
                    nidx = qc * 128
                    nc.gpsimd.dma_gather(
                        msgs[:, q * qc:(q + 1) * qc, :], tbl_grp[g][:],
                        idx_sb[:, q * qc * 8:(q + 1) * qc * 8],
                        nidx, nidx, ROWB,
                        single_packet=False, queue_num=q)
                maskseg = sb.tile([P, SEGC, P], F16, tag="maskseg")
                nc.vector.tensor_tensor(
                    out=maskseg[:],
                    in0=iota_sb[:].unsqueeze(1).to_broadcast([P, SEGC, P]),
                    in1=d8_sb[:].unsqueeze(2).to_broadcast([P, SEGC, P]),
                    op=mybir.AluOpType.is_equal)
                msgs16 = msgs[:].bitcast(F16)
                msgs8 = msgs[:].bitcast(F8)
                wex = sb.tile([P, SEGC, 132], F16, tag="wex")
                for su in range(SEGC // SUB):
                    k0 = su * SUB
                    sl = slice(k0, k0 + SUB)
                    mtp = ps.tile([P, SUB, P], F16, tag="mtp", space="PSUM")
                    for t in range(SUB):
                        nc.tensor.transpose(out=mtp[:, t, :],
                                            in_=maskseg[:, k0 + t, :],
                                            identity=id_sb[:])
                    mts8 = sb.tile([P, SUB, P], F16, tag="mts")
                    nc.scalar.copy(mts8[:], mtp[:])
                    adp = ps.tile([P, SUB, 4], F32, tag="adp", space="PSUM")
                    for t in range(SUB):
                        jj = max(chunk_meta[c0 + k0 + t][1], 0)
                        nc.tensor.matmul(adp[:, t, :], lhsT=mts8[:, t, :],
                                         rhs=ad_sb[:, jj, :], start=True,
                                         stop=True)
                    e1 = sb.tile([P, SUB, 4], F32, tag="e1")
                    nc.vector.tensor_tensor(out=e1[:],
                                            in0=msgs16[:, sl, 64:68],
                                            in1=adp[:],
                                            op=mybir.AluOpType.add)
                    e2 = sb.tile([P, SUB, 4], F32, tag="e2")
                    nc.vector.scalar_tensor_tensor(
                        out=e2[:], in0=e1[:], scalar=NEG, in1=e1[:],
                        op0=mybir.AluOpType.mult, op1=mybir.AluOpType.max)
                    nc.scalar.activation(wex[:, sl, 128:132], e2[:],
                                         mybir.ActivationFunctionType.Exp)
                    nc.vector.tensor_tensor(
                        out=wex[:, sl, 0:128].rearrange(
                            "p s (h c) -> p s h c", h=4),
                        in0=msgs8[:, sl, 0:128].rearrange(
                            "p s (h c) -> p s h c", h=4),
                        in1=wex[:, sl, 128:132].unsqueeze(3).to_broadcast(
                            [P, SUB, 4, 32]),
                        op=mybir.AluOpType.mult)
                    for t in range(SUB):
                        ci = c0 + k0 + t
                        g2, j, tt, first, last = chunk_meta[ci]
                        key = (g2, j) if j >= 0 else ("dummy", ci)
                        if first:
                            grp_ps[key] = ps.tile([P, 132], F32, tag="grp",
                                                  name="grp", space="PSUM")
                        gp = grp_ps[key]
                        nc.tensor.matmul(gp[:], lhsT=maskseg[:, k0 + t, :],
                                         rhs=wex[:, k0 + t, 0:132],
                                         start=first, stop=last)
                        if last and j >= 0:
                            nc.vector.tensor_tensor(
                                out=acc[:, j, :], in0=acc[:, j, :], in1=gp[:],
                                op=mybir.AluOpType.add)

        def elu_inplace(full_ap, nblk, width):
            EB = 4
            for q0 in range(0, nblk, EB):
                qn = min(EB, nblk - q0)
                ap = full_ap[:, q0:q0 + qn, :]
                shape = [P, EB, width]
                a = sb.tile(shape, F32, tag="elua")
                nc.scalar.activation(a[:, :qn, :], ap,
                                     mybir.ActivationFunctionType.Relu)
                bmin = sb.tile(shape, F32, tag="elub")
                nc.vector.tensor_scalar(out=bmin[:, :qn, :], in0=ap, scalar1=0.0,
                                        scalar2=None, op0=mybir.AluOpType.min)
                cc = sb.tile(shape, F32, tag="eluc")
                nc.scalar.activation(cc[:, :qn, :], bmin[:, :qn, :],
                                     mybir.ActivationFunctionType.Exp)
                nc.vector.scalar_tensor_tensor(
                    out=ap, in0=a[:, :qn, :], scalar=-1.0, in1=cc[:, :qn, :],
                    op0=mybir.AluOpType.add, op1=mybir.AluOpType.add)

        def finalize(l):
            rec = sb.tile([P, NBLK, 4], F32, tag="rec")
            nc.vector.reciprocal(out=rec[:], in_=acc[:, :, 128:132])
            nc.vector.tensor_tensor(
                out=yv[:].rearrange("p s (h c) -> p s h c", h=4),
                in0=acc[:, :, 0:128].rearrange("p s (h c) -> p s h c", h=4),
                in1=rec[:].unsqueeze(3).to_broadcast([P, NBLK, 4, 32]),
                op=mybir.AluOpType.mult)
            if l < 2:
                nc.vector.tensor_tensor(
                    out=yv[:], in0=yv[:],
                    in1=b_sb[l][:].unsqueeze(1).to_broadcast([P, NBLK, P]),
                    op=mybir.AluOpType.add)
                elu_inplace(yv, NBLK, P)

        for _rep in range(repeat):
          for l in range(3):
            if l == 0:
                w_transform_x()
            else:
                w_transform_y(l)
            build_tables(l)
            run_edges()
            finalize(l)

        pv = sb.tile([P, NBLK], F32, tag="pv")
        for q0 in range(0, NBLK, 16):
            qn = min(16, NBLK - q0)
            h3 = sb.tile([P, 16, HID], F32, tag="h3")
            nc.vector.tensor_reduce(
                out=h3[:, :qn, :],
                in_=yv[:, q0:q0 + qn, :].rearrange("p s (h c) -> p s c h", h=4),
                axis=mybir.AxisListType.X, op=mybir.AluOpType.add)
            nc.vector.tensor_scalar(out=h3[:, :qn, :], in0=h3[:, :qn, :],
                                    scalar1=0.25, scalar2=None,
                                    op0=mybir.AluOpType.mult)
            nc.vector.tensor_tensor(
                out=h3[:, :qn, :], in0=h3[:, :qn, :],
                in1=b3_sb[:].unsqueeze(1).to_broadcast([P, qn, HID]),
                op=mybir.AluOpType.add)
            elu_inplace(h3[:, :qn, :], qn, HID)
            tmp3 = sb.tile([P, 16, HID], F32, tag="tmp3")
            nc.vector.tensor_tensor(
                out=tmp3[:, :qn, :], in0=h3[:, :qn, :],
                in1=lw_sb[:].unsqueeze(1).to_broadcast([P, qn, HID]),
                op=mybir.AluOpType.mult)
            nc.vector.tensor_reduce(out=pv[:, q0:q0 + qn], in_=tmp3[:, :qn, :],
                                    axis=mybir.AxisListType.X,
                                    op=mybir.AluOpType.add)
        pool_ps = ps.tile([64, 1], F32, tag="big", space="PSUM")
        for s in range(NBLK):
            bps = sb.tile([P, 64], F32, tag="bps")
            nc.sync.dma_start(bps[:], bpool[:, s * 64:(s + 1) * 64])
            nc.tensor.matmul(pool_ps[:], lhsT=bps[:], rhs=pv[:, s:s + 1],
                             start=(s == 0), stop=(s == NBLK - 1))
        pool_sb = sb.tile([64, 1], F32, tag="poolsb")
        nc.scalar.copy(pool_sb[:], pool_ps[:])
        nc.sync.dma_start(out64[:], pool_sb[:])

    nc.compile()
    return nc


# ----------------------------------------------------------------------------
# host-side input construction
# ----------------------------------------------------------------------------
def make_inputs(sched, idx_arrs, d8_arrs, inputs, batch_counts=None):
    """Per-core in_maps from the raw problem inputs dict."""
    SH, NBLK, TROW = sched["SH"], sched["NBLK"], sched["TROW"]
    NROW = NBLK * 128
    x = np.asarray(inputs["x"], np.float32)
    N = x.shape[0]
    batch = np.asarray(inputs["batch"], np.int64)
    NGr = 64 if batch_counts is None else len(batch_counts)
    counts = np.bincount(batch, minlength=NGr).astype(np.float32)
    counts[counts == 0] = 1.0

    def rep(v, width=P):
        v = np.asarray(v, np.float32).reshape(1, -1)
        return np.tile(v, (P, 1))

    Ws = [np.asarray(inputs[k], np.float32).T.copy() for k in ("W1", "W2", "W3")]
    asr = [rep(np.asarray(inputs[k], np.float32).reshape(-1)).astype(np.float16)
           for k in ("a1s", "a2s", "a3s")]
    adr = [rep(np.asarray(inputs[k], np.float32).reshape(-1)).astype(np.float16)
           for k in ("a1d", "a2d", "a3d")]
    br = [rep(inputs["b1"]), rep(inputs["b2"])]
    b3r = rep(inputs["b3"])
    lwr = rep(np.asarray(inputs["lin_w"], np.float32).reshape(-1))
    iot = np.tile(np.arange(P, dtype=np.float32), (P, 1)).astype(np.float16)
    idf = np.eye(P, dtype=np.float16)
    idf32 = np.eye(P, dtype=np.float32)

    in_maps = []
    for c in range(NCORES):
        xs = np.zeros((TROW, P), np.float32)
        xs[0:SH] = x[c * SH:(c + 1) * SH]
        bp = np.zeros((NROW, 64), np.float32)
        b_loc = batch[c * SH:(c + 1) * SH]
        bp[np.arange(SH), b_loc] = 1.0 / counts[b_loc]
        m = {"xT": np.ascontiguousarray(xs.T), "idx16": idx_arrs[c],
             "d8col": d8_arrs[c],
             "b3rep": b3r, "lwrep": lwr, "iotarep": iot, "ident": idf,
             "ident32": idf32,
             "bpool": np.ascontiguousarray(
                 bp.reshape(NBLK, P, 64).transpose(1, 0, 2).reshape(P, NBLK * 64))}
        for l in range(3):
            m[f"WT{l}"] = Ws[l]
            m[f"asrep{l}"] = asr[l]
            m[f"adrep{l}"] = adr[l]
        for l in range(2):
            m[f"brep{l}"] = br[l]
        in_maps.append(m)
    return in_maps


# ----------------------------------------------------------------------------
# SPMD runner (modeled on bass2jax.run_bass_via_pjrt, with reusable executable)
# ----------------------------------------------------------------------------
def make_runner(nc, in_maps):
    import jax
    import jax.numpy as jnp
    from jax.sharding import Mesh, PartitionSpec
    from jax.experimental.shard_map import shard_map
    from concourse import bass2jax, mybir as mb

    bass2jax.install_neuronx_cc_hook()
    n_cores = len(in_maps)
    part_name = nc.partition_id_tensor.name if nc.partition_id_tensor else None
    in_names, out_names, out_avals, zero_outs = [], [], [], []
    for alloc in nc.m.functions[0].allocations:
        if not isinstance(alloc, mb.MemoryLocationSet):
            continue
        name = alloc.memorylocations[0].name
        if alloc.kind == "ExternalInput":
            if name != part_name:
                in_names.append(name)
        elif alloc.kind == "ExternalOutput":
            out_names.append(name)
            shape = tuple(alloc.tensor_shape)
            dtype = mb.dt.np(alloc.dtype)
            out_avals.append(jax.core.ShapedArray(shape, dtype))
            zero_outs.append(np.zeros(shape, dtype))
    n_params = len(in_names)
    all_names = in_names + out_names
    if part_name is not None:
        all_names = all_names + [part_name]

    def _body(*args):
        operands = list(args)
        if part_name is not None:
            operands.append(bass2jax.partition_id_tensor())
        outs = bass2jax._bass_exec_p.bind(
            *operands, out_avals=tuple(out_avals), in_names=tuple(all_names),
            out_names=tuple(out_names), lowering_input_output_aliases=(),
            sim_require_finite=False, sim_require_nnan=False, nc=nc)
        return tuple(outs)

    devices = jax.devices()[:n_cores]
    mesh = Mesh(np.asarray(devices), ("core",))
    in_specs = (PartitionSpec("core"),) * (n_params + len(out_names))
    out_specs = (PartitionSpec("core"),) * len(out_names)
    fn = jax.jit(shard_map(_body, mesh=mesh, in_specs=in_specs,
                           out_specs=out_specs, check_rep=False))
    concat_in = [np.concatenate([np.asarray(in_maps[c][nm])
                                 for c in range(n_cores)], axis=0)
                 for nm in in_names]
    concat_zeros = [np.zeros((n_cores * z.shape[0], *z.shape[1:]), z.dtype)
                    for z in zero_outs]
    dev_in = [jax.device_put(
        a, jax.sharding.NamedSharding(mesh, PartitionSpec("core")))
        for a in concat_in + concat_zeros]

    def run():
        outs = fn(*dev_in)
        outs = [np.asarray(o) for o in outs]
        return [
            {nm: outs[i].reshape(n_cores, *out_avals[i].shape)[c]
             for i, nm in enumerate(out_names)}
            for c in range(n_cores)]
    return run


def kernel(**inputs):
    """Full-input distributed GAT kernel; returns pooled [64] float32."""
    inputs = {k: np.asarray(v) for k, v in inputs.items()}
    N = inputs["x"].shape[0]
    sched, idx_arrs, d8_arrs = preprocess(inputs["edge_index"], N)
    nc = build_program(sched)
    in_maps = make_inputs(sched, idx_arrs, d8_arrs, inputs)
    run = make_runner(nc, in_maps)
    kernel.last_runner = run          # exposed for test.py timing
    kernel.last_inputs = inputs
    results = run()
    partial = sum(r["out64"][:, 0] for r in results)
    out = (partial + np.float32(inputs["lin_b"].reshape(-1)[0]))[:64]
    return out.astype(np.float32)


# revision 24
# speedup vs baseline: 1.9607x; 1.0005x over previous
"""Distributed GAT kernel for Trainium2 (8 NeuronCores), Bass/Tile. v2

Architecture (per layer):
  - node tables [TROW*8, 256B] rows = [h (128 f8e4) | as (4 f16) | pad],
    built per-core then replicated via 4 quarter AllGathers (overlap with
    edge compute of earlier quarters).
  - each core owns a dst shard; edges grouped by (src quarter g in 0..3,
    dst block j of 128 local dsts), chunked into 128-edge chunks (count =
    max over cores, so the SPMD program is identical on every core).
  - per segment (24 chunks): 4-queue dma_gather of 256B rows by src;
    ONE batched is_equal builds all 24 one-hot dst masks; per 8-chunk
    sub-batch: PE transposes masks into one PSUM bank, ONE Act copy out,
    per-chunk 4-col matmul gathers ad[dst]; attention e=exp(lrelu(as+ad))
    batched; weighted messages [a*h | e] in one 132-col tile; per-chunk
    scatter matmul accumulates [msgs|denom] per dst block in PSUM.
  - finalize: normalize by denom, +bias, ELU, transform with next W.
  - layer 3: mean heads, +b3, ELU, dot lin_w, pool via Bpool matmul ->
    [64] partial per core; host sums partials (+lin_b).
"""
import numpy as np
import ml_dtypes
from contextlib import ExitStack

import concourse.bacc as bacc
import concourse.bass as bass
import concourse.tile as tile
from concourse import mybir, bass_utils
from concourse.library_config import mlp

F16 = mybir.dt.float16
F32 = mybir.dt.float32
F8 = mybir.dt.float8e4
U8 = mybir.dt.uint8
I16 = mybir.dt.int16
NCORES = 8
P = 128
SEGC = 32          # chunks per gather segment
SUB = 8            # chunks per batched-op sub-batch
HEADS = 4
HID = 32
D1 = 128
NEG = 0.2
ROWB = 256         # table row bytes
F8NP = ml_dtypes.float8_e4m3


# ----------------------------------------------------------------------------
# host preprocessing
# ----------------------------------------------------------------------------
def preprocess(edge_index, N):
    """Build the core-independent schedule + per-core index arrays."""
    SH = N // NCORES
    assert SH * NCORES == N
    NBLK = (SH + 127) // 128          # dst blocks per core (last partial)
    TROW = NBLK * 128 + 128           # table rows/core: padded nodes + pad blk
    QROW = TROW // 4                  # rows per quarter (group)
    GSZ = NCORES * QROW               # rows per group table
    assert GSZ <= 32768

    E = edge_index.shape[1]
    src = np.concatenate([edge_index[0].astype(np.int64),
                          np.arange(N, dtype=np.int64)])
    dst = np.concatenate([edge_index[1].astype(np.int64),
                          np.arange(N, dtype=np.int64)])

    # per core: sort edges by (group, dst, srcrow); count per (g, block)
    per_core = []
    cnt = np.zeros((NCORES, 4, NBLK), np.int64)
    for c in range(NCORES):
        m = (dst // SH) == c
        s_c, d_c = src[m], dst[m] - c * SH
        sc, l = s_c // SH, s_c % SH
        g_c = np.minimum(l // QROW, 3)
        grow = sc * QROW + (l - g_c * QROW)
        order = np.lexsort((grow, d_c, g_c))
        s_row, d_l, g_c = grow[order], d_c[order], g_c[order]
        per_core.append((s_row, d_l, g_c))
        blk = d_l // 128
        np.add.at(cnt[c], (g_c, blk), 1)

    # chunks per (g, block) = cross-core max
    cpb = np.maximum(1, np.ceil(cnt.max(axis=0) / 128).astype(np.int64))

    # pad each group's chunk total to a multiple of SEGC (dummy block id -1)
    chunk_meta = []   # list of (g, blk, k, first, last) in program order
    for g in range(4):
        tot = 0
        for j in range(NBLK):
            k = int(cpb[g, j])
            for t in range(k):
                chunk_meta.append((g, j, t, t == 0, t == k - 1))
            tot += k
        padn = (-tot) % SEGC
        for t in range(padn):
            chunk_meta.append((g, -1, t, True, True))
    C = len(chunk_meta)

    # per-core edge->chunk-slot assignment
    idx_arrs, mask_arrs = [], []
    for c in range(NCORES):
        s_row, d_l, g_c = per_core[c]
        idx = np.zeros((C, 128), np.int64)          # idx within group table
        d8 = np.full((C, 128), 128, np.int64)       # pad slots kill the mask
        start = {}
        for ci, (g, j, t, fi, la) in enumerate(chunk_meta):
            if j >= 0 and t == 0:
                start[(g, j)] = ci
        for g in range(4):
            mg = g_c == g
            sg, dg = s_row[mg], d_l[mg]
            blocks = dg // 128
            for j in np.unique(blocks):
                mb = blocks == j
                rows, dl = sg[mb], dg[mb]
                c0 = start[(g, int(j))]
                n = len(rows)
                ch = np.arange(n) // 128
                sl = np.arange(n) % 128
                idx[c0 + ch, sl] = rows
                d8[c0 + ch, sl] = dl - int(j) * 128
        # dma_gather layout: idx i -> partition i%16, col i//16; replicate x8
        flat = idx.reshape(-1)
        il = np.zeros((16, C * 8), np.int16)
        ar = np.arange(C * 128)
        il[ar % 16, ar // 16] = flat.astype(np.int16)
        idx_arrs.append(np.tile(il, (8, 1)))
        # one-hot masks, fp8: mask [e, chunk, slot]; maskT [slot, chunk, e]
        onehot = d8[:, :, None] == np.arange(128)[None, None, :]  # [C, e, s]
        m8 = np.ascontiguousarray(
            onehot.transpose(1, 0, 2)).astype(F8NP).reshape(128, C * 128)
        mT8 = np.ascontiguousarray(
            onehot.transpose(2, 0, 1)).astype(F8NP).reshape(128, C * 128)
        mask_arrs.append((m8, mT8))
    sched = dict(SH=SH, NBLK=NBLK, TROW=TROW, QROW=QROW, GSZ=GSZ, C=C,
                 chunk_meta=chunk_meta)
    return sched, idx_arrs, mask_arrs


# ----------------------------------------------------------------------------
# device program
# ----------------------------------------------------------------------------
def build_program(sched, repeat=1, nq=4):
    SH, NBLK, TROW, C = sched["SH"], sched["NBLK"], sched["TROW"], sched["C"]
    QROW, GSZ = sched["QROW"], sched["GSZ"]
    chunk_meta = sched["chunk_meta"]
    NROW = NBLK * 128

    nc = bacc.Bacc("TRN2", target_bir_lowering=False, debug=False,
                   num_devices=NCORES, num_swdge_queues=max(nq, 1))

    xT = nc.dram_tensor("xT", [P, TROW], F32, kind="ExternalInput")
    idx16 = nc.dram_tensor("idx16", [P, C * 8], I16, kind="ExternalInput")
    mdram = nc.dram_tensor("mdram", [P, C * 128], F8, kind="ExternalInput")
    mTdram = nc.dram_tensor("mTdram", [P, C * 128], F8, kind="ExternalInput")
    WT = [nc.dram_tensor(f"WT{l}", [P, P], F32, kind="ExternalInput")
          for l in range(3)]
    asrep = [nc.dram_tensor(f"asrep{l}", [P, P], F16, kind="ExternalInput")
             for l in range(3)]
    adrep = [nc.dram_tensor(f"adrep{l}", [P, P], F16, kind="ExternalInput")
             for l in range(3)]
    brep = [nc.dram_tensor(f"brep{l}", [P, P], F32, kind="ExternalInput")
            for l in range(2)]
    b3rep = nc.dram_tensor("b3rep", [P, HID], F32, kind="ExternalInput")
    lwrep = nc.dram_tensor("lwrep", [P, HID], F32, kind="ExternalInput")
    ident32 = nc.dram_tensor("ident32", [P, P], F32, kind="ExternalInput")
    bpool = nc.dram_tensor("bpool", [P, NBLK * 64], F32, kind="ExternalInput")
    out64 = nc.dram_tensor("out64", [64, 1], F32, kind="ExternalOutput")

    with tile.TileContext(nc) as tc, ExitStack() as ctx:
        sb = ctx.enter_context(tc.tile_pool(name="sb", bufs=2))
        sbc = ctx.enter_context(tc.tile_pool(name="sbc", bufs=1))
        ps = ctx.enter_context(tc.tile_pool(name="ps", bufs=2, space="PSUM"))
        dr = ctx.enter_context(tc.tile_pool(name="dr", bufs=1, space="DRAM"))

        nc.gpsimd.load_library(mlp)

        id32_sb = sbc.tile([P, P], F32)
        nc.sync.dma_start(id32_sb[:], ident32[:])
        acc = sbc.tile([P, NBLK, 132], F32)
        ad_sb = sbc.tile([P, NBLK, 4], F16)
        as_node = sbc.tile([P, NBLK, 4], F16)
        hT = sbc.tile([P, TROW], F16)
        yv = acc[:, :, 0:128]

        tbl_in = dr.tile([TROW, ROWB], U8)
        tbl_grp = [None] * 4

        wt_sb = [sbc.tile([P, P], F32, tag=f"wt{l}", name=f"wt{l}") for l in range(3)]
        as_sb = [sbc.tile([P, P], F16, tag=f"asw{l}", name=f"asw{l}") for l in range(3)]
        adw_sb = [sbc.tile([P, P], F16, tag=f"adw{l}", name=f"adw{l}") for l in range(3)]
        b_sb = [sbc.tile([P, P], F32, tag=f"bb{l}", name=f"bb{l}") for l in range(2)]
        b3_sb = sbc.tile([P, HID], F32)
        lw_sb = sbc.tile([P, HID], F32)
        for l in range(3):
            nc.sync.dma_start(wt_sb[l][:], WT[l][:])
            nc.sync.dma_start(as_sb[l][:], asrep[l][:])
            nc.sync.dma_start(adw_sb[l][:], adrep[l][:])
        for l in range(2):
            nc.sync.dma_start(b_sb[l][:], brep[l][:])
        nc.sync.dma_start(b3_sb[:], b3rep[:])
        nc.sync.dma_start(lw_sb[:], lwrep[:])

        # zero the table pad rows once (content: h=0, as=0 -> harmless)
        zrow = sbc.tile([P, ROWB], U8)
        nc.vector.memset(zrow[:], 0.0)
        nc.sync.dma_start(
            tbl_in[NROW:TROW, :].rearrange("(s p) f -> p s f", p=P),
            zrow[:].unsqueeze(1))

        def w_transform_x():
            pos = 0
            while pos < TROW:
                w = min(512, TROW - pos)
                xs = sb.tile([P, 512], F32, tag="xs")
                nc.sync.dma_start(xs[:, :w], xT[:, pos:pos + w])
                hps = ps.tile([P, 512], F32, tag="big", space="PSUM")
                nc.tensor.matmul(hps[:, :w], lhsT=wt_sb[0][:],
                                 rhs=xs[:, :w], start=True, stop=True)
                nc.scalar.copy(hT[:, pos:pos + w], hps[:, :w])
                pos += w

        def w_transform_y(l):
            # fused: transpose yv (4 blocks) -> yT chunk -> W matmul -> hT
            for q in range((NBLK + 3) // 4):
                s0 = q * 4
                nb = min(4, NBLK - s0)
                ytc = sb.tile([P, 512], F32, tag="ytc")
                for t in range(nb):
                    tp = ps.tile([P, P], F32, tag="big", space="PSUM")
                    nc.tensor.transpose(out=tp[:], in_=yv[:, s0 + t, :],
                                        identity=id32_sb[:])
                    nc.scalar.copy(ytc[:, t * P:(t + 1) * P], tp[:])
                hps = ps.tile([P, 512], F32, tag="big", space="PSUM")
                nc.tensor.matmul(hps[:, :nb * P], lhsT=wt_sb[l][:],
                                 rhs=ytc[:, :nb * P], start=True, stop=True)
                nc.scalar.copy(hT[:, s0 * P:s0 * P + nb * P], hps[:, :nb * P])
            if NROW < TROW:
                nc.vector.memset(hT[:, NROW:TROW], 0.0)

        def build_tables(l):
            QB = 16
            for q0 in range(0, NBLK, QB):
                qn = min(QB, NBLK - q0)
                hb = sb.tile([P, QB, P], F16, tag="hnodeb")
                nc.sync.dma_start_transpose(
                    hb[:, :qn, :], hT[:, q0 * P:(q0 + qn) * P])
                for rep, red in ((as_sb[l], as_node), (adw_sb[l], ad_sb)):
                    tmp = sb.tile([P, QB, P], F16, tag="ashtmp")
                    nc.vector.tensor_tensor(
                        out=tmp[:, :qn, :], in0=hb[:, :qn, :],
                        in1=rep[:].unsqueeze(1).to_broadcast([P, qn, P]),
                        op=mybir.AluOpType.mult)
                    with nc.allow_low_precision(reason="32-term sums; f16 ok"):
                        nc.vector.tensor_reduce(
                            out=red[:, q0:q0 + qn, :],
                            in_=tmp[:, :qn, :].rearrange(
                                "p s (c h) -> p s h c", c=32),
                            axis=mybir.AxisListType.X, op=mybir.AluOpType.add)
                hb8 = sb.tile([P, QB, P], F8, tag="hnodeb8")
                nc.scalar.copy(hb8[:, :qn, :], hb[:, :qn, :])
                nc.sync.dma_start(
                    tbl_in[q0 * P:(q0 + qn) * P, 0:P].rearrange(
                        "(s p) f -> p s f", p=P),
                    hb8[:, :qn, :].bitcast(U8))
                nc.sync.dma_start(
                    tbl_in[:].bitcast(F16)[q0 * P:(q0 + qn) * P, 64:68].rearrange(
                        "(s p) f -> p s f", p=P), as_node[:, q0:q0 + qn, :])
            for q in range(4):
                tbl_grp[q] = dr.tile([GSZ, ROWB], U8, name=f"tblg{l}_{q}",
                                     addr_space="Shared")
                nc.gpsimd.collective_compute(
                    "AllGather", mybir.AluOpType.bypass,
                    replica_groups=[list(range(NCORES))],
                    ins=[tbl_in[q * QROW:(q + 1) * QROW, :].opt()],
                    outs=[tbl_grp[q][:].opt()])

        def run_edges():
            nc.vector.memset(acc[:, :, 0:128], 0.0)
            nc.vector.memset(acc[:, :, 128:132], 1e-30)
            grp_ps = {}
            nseg = C // SEGC
            for seg in range(nseg):
                c0 = seg * SEGC
                g = chunk_meta[c0][0]
                msgs = sb.tile([P, SEGC, ROWB], U8, tag="msgs")
                idx_sb = sb.tile([P, SEGC * 8], I16, tag="idxseg")
                nc.sync.dma_start(idx_sb[:], idx16[:, c0 * 8:(c0 + SEGC) * 8])
                qc = SEGC // nq
                for q in range(nq):
                    nidx = qc * 128
                    nc.gpsimd.dma_gather(
                        msgs[:, q * qc:(q + 1) * qc, :], tbl_grp[g][:],
                        idx_sb[:, q * qc * 8:(q + 1) * qc * 8],
                        nidx, nidx, ROWB,
                        single_packet=False, queue_num=q)
                mask8 = sb.tile([P, SEGC, P], F8, tag="mask8")
                nc.sync.dma_start(
                    mask8[:], mdram[:, c0 * 128:(c0 + SEGC) * 128].rearrange(
                        "p (s q) -> p s q", q=P))
                maskT8 = sb.tile([P, SEGC, P], F8, tag="maskT8")
                nc.sync.dma_start(
                    maskT8[:], mTdram[:, c0 * 128:(c0 + SEGC) * 128].rearrange(
                        "p (s q) -> p s q", q=P))
                msgs16 = msgs[:].bitcast(F16)
                msgs8 = msgs[:].bitcast(F8)
                hseg = sb.tile([P, SEGC, P], F16, tag="hseg")
                nc.scalar.copy(hseg[:], msgs8[:, :, 0:128])
                wex = sb.tile([P, SEGC, 132], F16, tag="wex")
                for su in range(SEGC // SUB):
                    k0 = su * SUB
                    sl = slice(k0, k0 + SUB)
                    adp = ps.tile([P, SUB, 4], F32, tag="adp", space="PSUM")
                    for t in range(SUB):
                        jj = max(chunk_meta[c0 + k0 + t][1], 0)
                        nc.tensor.matmul(adp[:, t, :],
                                         lhsT=maskT8[:, k0 + t, :],
                                         rhs=ad_sb[:, jj, :], start=True,
                                         stop=True)
                    e1 = sb.tile([P, SUB, 4], F32, tag="e1")
                    nc.vector.tensor_tensor(out=e1[:],
                                            in0=msgs16[:, sl, 64:68],
                                            in1=adp[:],
                                            op=mybir.AluOpType.add)
                    e2 = sb.tile([P, SUB, 4], F32, tag="e2")
                    nc.vector.scalar_tensor_tensor(
                        out=e2[:], in0=e1[:], scalar=NEG, in1=e1[:],
                        op0=mybir.AluOpType.mult, op1=mybir.AluOpType.max)
                    nc.scalar.activation(wex[:, sl, 128:132], e2[:],
                                         mybir.ActivationFunctionType.Exp)
                    nc.vector.tensor_tensor(
                        out=wex[:, sl, 0:128].rearrange(
                            "p s (c h) -> p s c h", c=32),
                        in0=hseg[:, sl, :].rearrange(
                            "p s (c h) -> p s c h", c=32),
                        in1=wex[:, sl, 128:132].unsqueeze(2).to_broadcast(
                            [P, SUB, 32, 4]),
                        op=mybir.AluOpType.mult)
                    for t in range(SUB):
                        ci = c0 + k0 + t
                        g2, j, tt, first, last = chunk_meta[ci]
                        key = (g2, j) if j >= 0 else ("dummy", ci)
                        if first:
                            grp_ps[key] = ps.tile([P, 132], F32, tag="grp",
                                                  name="grp", space="PSUM")
                        gp = grp_ps[key]
                        nc.tensor.matmul(gp[:], lhsT=mask8[:, k0 + t, :],
                                         rhs=wex[:, k0 + t, 0:132],
                                         start=first, stop=last)
                        if last and j >= 0:
                            nc.vector.tensor_tensor(
                                out=acc[:, j, :], in0=acc[:, j, :], in1=gp[:],
                                op=mybir.AluOpType.add)

        def elu_inplace(full_ap, nblk, width):
            EB = 4
            for q0 in range(0, nblk, EB):
                qn = min(EB, nblk - q0)
                ap = full_ap[:, q0:q0 + qn, :]
                shape = [P, EB, width]
                a = sb.tile(shape, F32, tag="elua")
                nc.scalar.activation(a[:, :qn, :], ap,
                                     mybir.ActivationFunctionType.Relu)
                bmin = sb.tile(shape, F32, tag="elub")
                nc.vector.tensor_scalar(out=bmin[:, :qn, :], in0=ap, scalar1=0.0,
                                        scalar2=None, op0=mybir.AluOpType.min)
                cc = sb.tile(shape, F32, tag="eluc")
                nc.scalar.activation(cc[:, :qn, :], bmin[:, :qn, :],
                                     mybir.ActivationFunctionType.Exp)
                nc.vector.scalar_tensor_tensor(
                    out=ap, in0=a[:, :qn, :], scalar=-1.0, in1=cc[:, :qn, :],
                    op0=mybir.AluOpType.add, op1=mybir.AluOpType.add)

        def finalize(l):
            rec = sb.tile([P, NBLK, 4], F32, tag="rec")
            nc.vector.reciprocal(out=rec[:], in_=acc[:, :, 128:132])
            nc.vector.tensor_tensor(
                out=yv[:].rearrange("p s (c h) -> p s c h", c=32),
                in0=acc[:, :, 0:128].rearrange("p s (c h) -> p s c h", c=32),
                in1=rec[:].unsqueeze(2).to_broadcast([P, NBLK, 32, 4]),
                op=mybir.AluOpType.mult)
            if l < 2:
                nc.vector.tensor_tensor(
                    out=yv[:], in0=yv[:],
                    in1=b_sb[l][:].unsqueeze(1).to_broadcast([P, NBLK, P]),
                    op=mybir.AluOpType.add)
                elu_inplace(yv, NBLK, P)

        for _rep in range(repeat):
          for l in range(3):
            if l == 0:
                w_transform_x()
            else:
                w_transform_y(l)
            build_tables(l)
            run_edges()
            finalize(l)

        pv = sb.tile([P, NBLK], F32, tag="pv")
        for q0 in range(0, NBLK, 16):
            qn = min(16, NBLK - q0)
            h3 = sb.tile([P, 16, HID], F32, tag="h3")
            nc.vector.tensor_reduce(
                out=h3[:, :qn, :],
                in_=yv[:, q0:q0 + qn, :].rearrange("p s (c h) -> p s c h", c=32),
                axis=mybir.AxisListType.X, op=mybir.AluOpType.add)
            nc.vector.tensor_scalar(out=h3[:, :qn, :], in0=h3[:, :qn, :],
                                    scalar1=0.25, scalar2=None,
                                    op0=mybir.AluOpType.mult)
            nc.vector.tensor_tensor(
                out=h3[:, :qn, :], in0=h3[:, :qn, :],
                in1=b3_sb[:].unsqueeze(1).to_broadcast([P, qn, HID]),
                op=mybir.AluOpType.add)
            elu_inplace(h3[:, :qn, :], qn, HID)
            tmp3 = sb.tile([P, 16, HID], F32, tag="tmp3")
            nc.vector.tensor_tensor(
                out=tmp3[:, :qn, :], in0=h3[:, :qn, :],
                in1=lw_sb[:].unsqueeze(1).to_broadcast([P, qn, HID]),
                op=mybir.AluOpType.mult)
            nc.vector.tensor_reduce(out=pv[:, q0:q0 + qn], in_=tmp3[:, :qn, :],
                                    axis=mybir.AxisListType.X,
                                    op=mybir.AluOpType.add)
        pool_ps = ps.tile([64, 1], F32, tag="big", space="PSUM")
        for s in range(NBLK):
            bps = sb.tile([P, 64], F32, tag="bps")
            nc.sync.dma_start(bps[:], bpool[:, s * 64:(s + 1) * 64])
            nc.tensor.matmul(pool_ps[:], lhsT=bps[:], rhs=pv[:, s:s + 1],
                             start=(s == 0), stop=(s == NBLK - 1))
        pool_sb = sb.tile([64, 1], F32, tag="poolsb")
        nc.scalar.copy(pool_sb[:], pool_ps[:])
        nc.sync.dma_start(out64[:], pool_sb[:])

    nc.compile()
    return nc


# ----------------------------------------------------------------------------
# host-side input construction
# ----------------------------------------------------------------------------
def make_inputs(sched, idx_arrs, mask_arrs, inputs, batch_counts=None):
    """Per-core in_maps from the raw problem inputs dict.

    Features are re-laid-out c-major (f_new = c*4 + h) so the per-head
    attention broadcast lands on a packed last dim (DVE 2x mode).
    """
    SH, NBLK, TROW = sched["SH"], sched["NBLK"], sched["TROW"]
    NROW = NBLK * 128
    x = np.asarray(inputs["x"], np.float32)
    N = x.shape[0]
    batch = np.asarray(inputs["batch"], np.int64)
    NGr = 64 if batch_counts is None else len(batch_counts)
    counts = np.bincount(batch, minlength=NGr).astype(np.float32)
    counts[counts == 0] = 1.0

    pf = np.arange(P)
    perm = (pf % HEADS) * HID + pf // HEADS   # old h-major idx at new position

    def rep(v, width=P):
        v = np.asarray(v, np.float32).reshape(1, -1)
        return np.tile(v, (P, 1))

    W1 = np.asarray(inputs["W1"], np.float32)[perm, :]
    W2 = np.asarray(inputs["W2"], np.float32)[perm][:, perm]
    W3 = np.asarray(inputs["W3"], np.float32)[perm][:, perm]
    Ws = [np.ascontiguousarray(W.T) for W in (W1, W2, W3)]
    asr = [rep(np.asarray(inputs[k], np.float32).reshape(-1)[perm]).astype(
        np.float16) for k in ("a1s", "a2s", "a3s")]
    adr = [rep(np.asarray(inputs[k], np.float32).reshape(-1)[perm]).astype(
        np.float16) for k in ("a1d", "a2d", "a3d")]
    br = [rep(np.asarray(inputs["b1"], np.float32)[perm]),
          rep(np.asarray(inputs["b2"], np.float32)[perm])]
    b3r = rep(inputs["b3"])
    lwr = rep(np.asarray(inputs["lin_w"], np.float32).reshape(-1))
    idf32 = np.eye(P, dtype=np.float32)

    in_maps = []
    for c in range(NCORES):
        xs = np.zeros((TROW, P), np.float32)
        xs[0:SH] = x[c * SH:(c + 1) * SH]
        bp = np.zeros((NROW, 64), np.float32)
        b_loc = batch[c * SH:(c + 1) * SH]
        bp[np.arange(SH), b_loc] = 1.0 / counts[b_loc]
        m = {"xT": np.ascontiguousarray(xs.T), "idx16": idx_arrs[c],
             "mdram": mask_arrs[c][0], "mTdram": mask_arrs[c][1],
             "b3rep": b3r, "lwrep": lwr,
             "ident32": idf32,
             "bpool": np.ascontiguousarray(
                 bp.reshape(NBLK, P, 64).transpose(1, 0, 2).reshape(P, NBLK * 64))}
        for l in range(3):
            m[f"WT{l}"] = Ws[l]
            m[f"asrep{l}"] = asr[l]
            m[f"adrep{l}"] = adr[l]
        for l in range(2):
            m[f"brep{l}"] = br[l]
        in_maps.append(m)
    return in_maps


# ----------------------------------------------------------------------------
# SPMD runner (modeled on bass2jax.run_bass_via_pjrt, with reusable executable)
# ----------------------------------------------------------------------------
def make_runner(nc, in_maps):
    import jax
    import jax.numpy as jnp
    from jax.sharding import Mesh, PartitionSpec
    from jax.experimental.shard_map import shard_map
    from concourse import bass2jax, mybir as mb

    bass2jax.install_neuronx_cc_hook()
    n_cores = len(in_maps)
    part_name = nc.partition_id_tensor.name if nc.partition_id_tensor else None
    in_names, out_names, out_avals, zero_outs = [], [], [], []
    for alloc in nc.m.functions[0].allocations:
        if not isinstance(alloc, mb.MemoryLocationSet):
            continue
        name = alloc.memorylocations[0].name
        if alloc.kind == "ExternalInput":
            if name != part_name:
                in_names.append(name)
        elif alloc.kind == "ExternalOutput":
            out_names.append(name)
            shape = tuple(alloc.tensor_shape)
            dtype = mb.dt.np(alloc.dtype)
            out_avals.append(jax.core.ShapedArray(shape, dtype))
            zero_outs.append(np.zeros(shape, dtype))
    n_params = len(in_names)
    all_names = in_names + out_names
    if part_name is not None:
        all_names = all_names + [part_name]

    def _body(*args):
        operands = list(args)
        if part_name is not None:
            operands.append(bass2jax.partition_id_tensor())
        outs = bass2jax._bass_exec_p.bind(
            *operands, out_avals=tuple(out_avals), in_names=tuple(all_names),
            out_names=tuple(out_names), lowering_input_output_aliases=(),
            sim_require_finite=False, sim_require_nnan=False, nc=nc)
        return tuple(outs)

    devices = jax.devices()[:n_cores]
    mesh = Mesh(np.asarray(devices), ("core",))
    in_specs = (PartitionSpec("core"),) * (n_params + len(out_names))
    out_specs = (PartitionSpec("core"),) * len(out_names)
    fn = jax.jit(shard_map(_body, mesh=mesh, in_specs=in_specs,
                           out_specs=out_specs, check_rep=False))
    concat_in = [np.concatenate([np.asarray(in_maps[c][nm])
                                 for c in range(n_cores)], axis=0)
                 for nm in in_names]
    concat_zeros = [np.zeros((n_cores * z.shape[0], *z.shape[1:]), z.dtype)
                    for z in zero_outs]
    dev_in = [jax.device_put(
        a, jax.sharding.NamedSharding(mesh, PartitionSpec("core")))
        for a in concat_in + concat_zeros]

    def run():
        outs = fn(*dev_in)
        outs = [np.asarray(o) for o in outs]
        return [
            {nm: outs[i].reshape(n_cores, *out_avals[i].shape)[c]
             for i, nm in enumerate(out_names)}
            for c in range(n_cores)]
    return run


def kernel(**inputs):
    """Full-input distributed GAT kernel; returns pooled [64] float32."""
    inputs = {k: np.asarray(v) for k, v in inputs.items()}
    N = inputs["x"].shape[0]
    sched, idx_arrs, d8_arrs = preprocess(inputs["edge_index"], N)
    nc = build_program(sched)
    in_maps = make_inputs(sched, idx_arrs, d8_arrs, inputs)
    run = make_runner(nc, in_maps)
    kernel.last_runner = run          # exposed for test.py timing
    kernel.last_inputs = inputs
    results = run()
    partial = sum(r["out64"][:, 0] for r in results)
    out = (partial + np.float32(inputs["lin_b"].reshape(-1)[0]))[:64]
    return out.astype(np.float32)


# revision 27
# speedup vs baseline: 2.8279x; 1.4422x over previous
"""Distributed GAT kernel for Trainium2 (8 NeuronCores), Bass/Tile. v2

Architecture (per layer):
  - node tables [TROW*8, 256B] rows = [h (128 f8e4) | as (4 f16) | pad],
    built per-core then replicated via 4 quarter AllGathers (overlap with
    edge compute of earlier quarters).
  - each core owns a dst shard; edges grouped by (src quarter g in 0..3,
    dst block j of 128 local dsts), chunked into 128-edge chunks (count =
    max over cores, so the SPMD program is identical on every core).
  - per segment (24 chunks): 4-queue dma_gather of 256B rows by src;
    ONE batched is_equal builds all 24 one-hot dst masks; per 8-chunk
    sub-batch: PE transposes masks into one PSUM bank, ONE Act copy out,
    per-chunk 4-col matmul gathers ad[dst]; attention e=exp(lrelu(as+ad))
    batched; weighted messages [a*h | e] in one 132-col tile; per-chunk
    scatter matmul accumulates [msgs|denom] per dst block in PSUM.
  - finalize: normalize by denom, +bias, ELU, transform with next W.
  - layer 3: mean heads, +b3, ELU, dot lin_w, pool via Bpool matmul ->
    [64] partial per core; host sums partials (+lin_b).
"""
import numpy as np
import ml_dtypes
from contextlib import ExitStack

import concourse.bacc as bacc
import concourse.bass as bass
import concourse.tile as tile
from concourse import mybir, bass_utils
from concourse.library_config import mlp

F16 = mybir.dt.float16
F32 = mybir.dt.float32
F8 = mybir.dt.float8e4
U8 = mybir.dt.uint8
I16 = mybir.dt.int16
NCORES = 8
P = 128
SEGC = 32          # chunks per gather segment
SUB = 8            # chunks per batched-op sub-batch
HEADS = 4
HID = 32
D1 = 128
NEG = 0.2
ROWB = 256         # table row bytes
F8NP = ml_dtypes.float8_e4m3


# ----------------------------------------------------------------------------
# host preprocessing
# ----------------------------------------------------------------------------
def preprocess(edge_index, N):
    """Build the core-independent schedule + per-core index arrays."""
    SH = N // NCORES
    assert SH * NCORES == N
    NBLK = (SH + 127) // 128          # dst blocks per core (last partial)
    TROW = NBLK * 128 + 128           # table rows/core: padded nodes + pad blk
    QROW = TROW // 4                  # rows per quarter (group)
    GSZ = NCORES * QROW               # rows per group table
    assert GSZ <= 32768

    E = edge_index.shape[1]
    src = np.concatenate([edge_index[0].astype(np.int64),
                          np.arange(N, dtype=np.int64)])
    dst = np.concatenate([edge_index[1].astype(np.int64),
                          np.arange(N, dtype=np.int64)])

    # per core: sort edges by (group, dst, srcrow); count per (g, block)
    per_core = []
    cnt = np.zeros((NCORES, 4, NBLK), np.int64)
    for c in range(NCORES):
        m = (dst // SH) == c
        s_c, d_c = src[m], dst[m] - c * SH
        sc, l = s_c // SH, s_c % SH
        g_c = np.minimum(l // QROW, 3)
        grow = sc * QROW + (l - g_c * QROW)
        order = np.lexsort((grow, d_c, g_c))
        s_row, d_l, g_c = grow[order], d_c[order], g_c[order]
        per_core.append((s_row, d_l, g_c))
        blk = d_l // 128
        np.add.at(cnt[c], (g_c, blk), 1)

    # chunks per (g, block) = cross-core max
    cpb = np.maximum(1, np.ceil(cnt.max(axis=0) / 128).astype(np.int64))

    # pad each group's chunk total to a multiple of SEGC (dummy block id -1)
    chunk_meta = []   # list of (g, blk, k, first, last) in program order
    for g in range(4):
        tot = 0
        for j in range(NBLK):
            k = int(cpb[g, j])
            for t in range(k):
                chunk_meta.append((g, j, t, t == 0, t == k - 1))
            tot += k
        padn = (-tot) % SEGC
        for t in range(padn):
            chunk_meta.append((g, -1, t, True, True))
    C = len(chunk_meta)

    # per-core edge->chunk-slot assignment
    idx_arrs, mask_arrs = [], []
    for c in range(NCORES):
        s_row, d_l, g_c = per_core[c]
        idx = np.zeros((C, 128), np.int64)          # idx within group table
        d8 = np.full((C, 128), 128, np.int64)       # pad slots kill the mask
        start = {}
        for ci, (g, j, t, fi, la) in enumerate(chunk_meta):
            if j >= 0 and t == 0:
                start[(g, j)] = ci
        for g in range(4):
            mg = g_c == g
            sg, dg = s_row[mg], d_l[mg]
            blocks = dg // 128
            for j in np.unique(blocks):
                mb = blocks == j
                rows, dl = sg[mb], dg[mb]
                c0 = start[(g, int(j))]
                n = len(rows)
                ch = np.arange(n) // 128
                sl = np.arange(n) % 128
                idx[c0 + ch, sl] = rows
                d8[c0 + ch, sl] = dl - int(j) * 128
        # dma_gather layout: idx i -> partition i%16, col i//16; replicate x8
        flat = idx.reshape(-1)
        il = np.zeros((16, C * 8), np.int16)
        ar = np.arange(C * 128)
        il[ar % 16, ar // 16] = flat.astype(np.int16)
        idx_arrs.append(np.tile(il, (8, 1)))
        # one-hot masks, fp8: mask [e, chunk, slot]; maskT [slot, chunk, e]
        onehot = d8[:, :, None] == np.arange(128)[None, None, :]  # [C, e, s]
        m8 = np.ascontiguousarray(
            onehot.transpose(1, 0, 2)).astype(F8NP).reshape(128, C * 128)
        mT8 = np.ascontiguousarray(
            onehot.transpose(2, 0, 1)).astype(F8NP).reshape(128, C * 128)
        mask_arrs.append((m8, mT8))
    sched = dict(SH=SH, NBLK=NBLK, TROW=TROW, QROW=QROW, GSZ=GSZ, C=C,
                 chunk_meta=chunk_meta)
    return sched, idx_arrs, mask_arrs


# ----------------------------------------------------------------------------
# device program
# ----------------------------------------------------------------------------
def build_program(sched, repeat=1, nq=4):
    SH, NBLK, TROW, C = sched["SH"], sched["NBLK"], sched["TROW"], sched["C"]
    QROW, GSZ = sched["QROW"], sched["GSZ"]
    chunk_meta = sched["chunk_meta"]
    NROW = NBLK * 128

    nc = bacc.Bacc("TRN2", target_bir_lowering=False, debug=False,
                   num_devices=NCORES, num_swdge_queues=max(nq, 1))

    xT = nc.dram_tensor("xT", [P, TROW], F32, kind="ExternalInput")
    idx16 = nc.dram_tensor("idx16", [P, C * 8], I16, kind="ExternalInput")
    mdram = nc.dram_tensor("mdram", [P, C * 128], F8, kind="ExternalInput")
    mTdram = nc.dram_tensor("mTdram", [P, C * 128], F8, kind="ExternalInput")
    WT = [nc.dram_tensor(f"WT{l}", [P, P], F32, kind="ExternalInput")
          for l in range(3)]
    asrep = [nc.dram_tensor(f"asrep{l}", [P, P], F16, kind="ExternalInput")
             for l in range(3)]
    adrep = [nc.dram_tensor(f"adrep{l}", [P, P], F16, kind="ExternalInput")
             for l in range(3)]
    brep = [nc.dram_tensor(f"brep{l}", [P, P], F32, kind="ExternalInput")
            for l in range(2)]
    b3rep = nc.dram_tensor("b3rep", [P, HID], F32, kind="ExternalInput")
    lwrep = nc.dram_tensor("lwrep", [P, HID], F32, kind="ExternalInput")
    ident32 = nc.dram_tensor("ident32", [P, P], F32, kind="ExternalInput")
    bpool = nc.dram_tensor("bpool", [P, NBLK * 64], F32, kind="ExternalInput")
    out64 = nc.dram_tensor("out64", [64, 1], F32, kind="ExternalOutput")

    with tile.TileContext(nc) as tc, ExitStack() as ctx:
        sb = ctx.enter_context(tc.tile_pool(name="sb", bufs=2))
        sbg = ctx.enter_context(tc.tile_pool(name="sbg", bufs=3))
        sbc = ctx.enter_context(tc.tile_pool(name="sbc", bufs=1))
        ps = ctx.enter_context(tc.tile_pool(name="ps", bufs=2, space="PSUM"))
        dr = ctx.enter_context(tc.tile_pool(name="dr", bufs=1, space="DRAM"))

        nc.gpsimd.load_library(mlp)

        id32_sb = sbc.tile([P, P], F32)
        nc.sync.dma_start(id32_sb[:], ident32[:])
        acc = sbc.tile([P, NBLK, 132], F32)
        ad_sb = sbc.tile([P, NBLK, 4], F16)
        as_node = sbc.tile([P, NBLK, 4], F16)
        hT = sbc.tile([P, TROW], F16)
        yv = acc[:, :, 0:128]

        tbl_in = dr.tile([TROW, ROWB], U8)
        tbl_grp = [None] * 4

        wt_sb = [sbc.tile([P, P], F32, tag=f"wt{l}", name=f"wt{l}") for l in range(3)]
        as_sb = [sbc.tile([P, P], F16, tag=f"asw{l}", name=f"asw{l}") for l in range(3)]
        adw_sb = [sbc.tile([P, P], F16, tag=f"adw{l}", name=f"adw{l}") for l in range(3)]
        b_sb = [sbc.tile([P, P], F32, tag=f"bb{l}", name=f"bb{l}") for l in range(2)]
        b3_sb = sbc.tile([P, HID], F32)
        lw_sb = sbc.tile([P, HID], F32)
        for l in range(3):
            nc.sync.dma_start(wt_sb[l][:], WT[l][:])
            nc.sync.dma_start(as_sb[l][:], asrep[l][:])
            nc.sync.dma_start(adw_sb[l][:], adrep[l][:])
        for l in range(2):
            nc.sync.dma_start(b_sb[l][:], brep[l][:])
        nc.sync.dma_start(b3_sb[:], b3rep[:])
        nc.sync.dma_start(lw_sb[:], lwrep[:])

        # zero the table pad rows once (content: h=0, as=0 -> harmless)
        zrow = sbc.tile([P, ROWB], U8)
        nc.vector.memset(zrow[:], 0.0)
        nc.sync.dma_start(
            tbl_in[NROW:TROW, :].rearrange("(s p) f -> p s f", p=P),
            zrow[:].unsqueeze(1))

        def w_transform_x():
            pos = 0
            while pos < TROW:
                w = min(512, TROW - pos)
                xs = sb.tile([P, 512], F32, tag="xs")
                nc.sync.dma_start(xs[:, :w], xT[:, pos:pos + w])
                hps = ps.tile([P, 512], F32, tag="big", space="PSUM")
                nc.tensor.matmul(hps[:, :w], lhsT=wt_sb[0][:],
                                 rhs=xs[:, :w], start=True, stop=True)
                nc.scalar.copy(hT[:, pos:pos + w], hps[:, :w])
                pos += w

        def w_transform_y(l):
            # fused: transpose yv (4 blocks) -> yT chunk -> W matmul -> hT
            for q in range((NBLK + 3) // 4):
                s0 = q * 4
                nb = min(4, NBLK - s0)
                ytc = sb.tile([P, 512], F32, tag="ytc")
                for t in range(nb):
                    tp = ps.tile([P, P], F32, tag="big", space="PSUM")
                    nc.tensor.transpose(out=tp[:], in_=yv[:, s0 + t, :],
                                        identity=id32_sb[:])
                    nc.scalar.copy(ytc[:, t * P:(t + 1) * P], tp[:])
                hps = ps.tile([P, 512], F32, tag="big", space="PSUM")
                nc.tensor.matmul(hps[:, :nb * P], lhsT=wt_sb[l][:],
                                 rhs=ytc[:, :nb * P], start=True, stop=True)
                nc.scalar.copy(hT[:, s0 * P:s0 * P + nb * P], hps[:, :nb * P])
            if NROW < TROW:
                nc.vector.memset(hT[:, NROW:TROW], 0.0)

        def build_tables(l):
            QB = 13
            for q0 in range(0, NBLK, QB):
                qn = min(QB, NBLK - q0)
                hb = sb.tile([P, QB, P], F16, tag="hnodeb")
                nc.sync.dma_start_transpose(
                    hb[:, :qn, :], hT[:, q0 * P:(q0 + qn) * P])
                for rep, red in ((as_sb[l], as_node), (adw_sb[l], ad_sb)):
                    tmp = sb.tile([P, QB, P], F16, tag="ashtmp")
                    nc.vector.tensor_tensor(
                        out=tmp[:, :qn, :], in0=hb[:, :qn, :],
                        in1=rep[:].unsqueeze(1).to_broadcast([P, qn, P]),
                        op=mybir.AluOpType.mult)
                    with nc.allow_low_precision(reason="32-term sums; f16 ok"):
                        nc.vector.tensor_reduce(
                            out=red[:, q0:q0 + qn, :],
                            in_=tmp[:, :qn, :].rearrange(
                                "p s (c h) -> p s h c", c=32),
                            axis=mybir.AxisListType.X, op=mybir.AluOpType.add)
                hb8 = sb.tile([P, QB, P], F8, tag="hnodeb8")
                nc.scalar.copy(hb8[:, :qn, :], hb[:, :qn, :])
                nc.sync.dma_start(
                    tbl_in[q0 * P:(q0 + qn) * P, 0:P].rearrange(
                        "(s p) f -> p s f", p=P),
                    hb8[:, :qn, :].bitcast(U8))
                nc.sync.dma_start(
                    tbl_in[:].bitcast(F16)[q0 * P:(q0 + qn) * P, 64:68].rearrange(
                        "(s p) f -> p s f", p=P), as_node[:, q0:q0 + qn, :])
            for q in range(4):
                tbl_grp[q] = dr.tile([GSZ, ROWB], U8, name=f"tblg{l}_{q}",
                                     addr_space="Shared")
                nc.gpsimd.collective_compute(
                    "AllGather", mybir.AluOpType.bypass,
                    replica_groups=[list(range(NCORES))],
                    ins=[tbl_in[q * QROW:(q + 1) * QROW, :].opt()],
                    outs=[tbl_grp[q][:].opt()])

        def run_edges():
            nc.vector.memset(acc[:, :, 0:128], 0.0)
            nc.vector.memset(acc[:, :, 128:132], 1e-30)
            grp_ps = {}
            nseg = C // SEGC
            for seg in range(nseg):
                c0 = seg * SEGC
                g = chunk_meta[c0][0]
                msgs = sbg.tile([P, SEGC, ROWB], U8, tag="msgs")
                idx_sb = sbg.tile([P, SEGC * 8], I16, tag="idxseg")
                nc.sync.dma_start(idx_sb[:], idx16[:, c0 * 8:(c0 + SEGC) * 8])
                qc = SEGC // nq
                for q in range(nq):
                    nidx = qc * 128
                    nc.gpsimd.dma_gather(
                        msgs[:, q * qc:(q + 1) * qc, :], tbl_grp[g][:],
                        idx_sb[:, q * qc * 8:(q + 1) * qc * 8],
                        nidx, nidx, ROWB,
                        single_packet=False, queue_num=q)
                mask8 = sb.tile([P, SEGC, P], F8, tag="mask8")
                nc.sync.dma_start(
                    mask8[:], mdram[:, c0 * 128:(c0 + SEGC) * 128].rearrange(
                        "p (s q) -> p s q", q=P))
                maskT8 = sb.tile([P, SEGC, P], F8, tag="maskT8")
                nc.sync.dma_start(
                    maskT8[:], mTdram[:, c0 * 128:(c0 + SEGC) * 128].rearrange(
                        "p (s q) -> p s q", q=P))
                msgs16 = msgs[:].bitcast(F16)
                msgs8 = msgs[:].bitcast(F8)
                hseg = sb.tile([P, SEGC, P], F16, tag="hseg")
                nc.scalar.copy(hseg[:], msgs8[:, :, 0:128])
                wex = sb.tile([P, SEGC, 132], F16, tag="wex")
                for su in range(SEGC // SUB):
                    k0 = su * SUB
                    sl = slice(k0, k0 + SUB)
                    adp = ps.tile([P, SUB, 4], F32, tag="adp", space="PSUM")
                    for t in range(SUB):
                        jj = max(chunk_meta[c0 + k0 + t][1], 0)
                        nc.tensor.matmul(adp[:, t, :],
                                         lhsT=maskT8[:, k0 + t, :],
                                         rhs=ad_sb[:, jj, :], start=True,
                                         stop=True)
                    e1 = sb.tile([P, SUB, 4], F32, tag="e1")
                    nc.vector.tensor_tensor(out=e1[:],
                                            in0=msgs16[:, sl, 64:68],
                                            in1=adp[:],
                                            op=mybir.AluOpType.add)
                    e2 = sb.tile([P, SUB, 4], F32, tag="e2")
                    nc.vector.scalar_tensor_tensor(
                        out=e2[:], in0=e1[:], scalar=NEG, in1=e1[:],
                        op0=mybir.AluOpType.mult, op1=mybir.AluOpType.max)
                    nc.scalar.activation(wex[:, sl, 128:132], e2[:],
                                         mybir.ActivationFunctionType.Exp)
                    nc.vector.tensor_tensor(
                        out=wex[:, sl, 0:128].rearrange(
                            "p s (c h) -> p s c h", c=32),
                        in0=hseg[:, sl, :].rearrange(
                            "p s (c h) -> p s c h", c=32),
                        in1=wex[:, sl, 128:132].unsqueeze(2).to_broadcast(
                            [P, SUB, 32, 4]),
                        op=mybir.AluOpType.mult)
                    for t in range(SUB):
                        ci = c0 + k0 + t
                        g2, j, tt, first, last = chunk_meta[ci]
                        key = (g2, j) if j >= 0 else ("dummy", ci)
                        if first:
                            grp_ps[key] = ps.tile([P, 132], F32, tag="grp",
                                                  name="grp", space="PSUM")
                        gp = grp_ps[key]
                        nc.tensor.matmul(gp[:], lhsT=mask8[:, k0 + t, :],
                                         rhs=wex[:, k0 + t, 0:132],
                                         start=first, stop=last)
                        if last and j >= 0:
                            nc.vector.tensor_tensor(
                                out=acc[:, j, :], in0=acc[:, j, :], in1=gp[:],
                                op=mybir.AluOpType.add)

        def elu_inplace(full_ap, nblk, width):
            EB = 4
            for q0 in range(0, nblk, EB):
                qn = min(EB, nblk - q0)
                ap = full_ap[:, q0:q0 + qn, :]
                shape = [P, EB, width]
                a = sb.tile(shape, F32, tag="elua")
                nc.scalar.activation(a[:, :qn, :], ap,
                                     mybir.ActivationFunctionType.Relu)
                bmin = sb.tile(shape, F32, tag="elub")
                nc.vector.tensor_scalar(out=bmin[:, :qn, :], in0=ap, scalar1=0.0,
                                        scalar2=None, op0=mybir.AluOpType.min)
                cc = sb.tile(shape, F32, tag="eluc")
                nc.scalar.activation(cc[:, :qn, :], bmin[:, :qn, :],
                                     mybir.ActivationFunctionType.Exp)
                nc.vector.scalar_tensor_tensor(
                    out=ap, in0=a[:, :qn, :], scalar=-1.0, in1=cc[:, :qn, :],
                    op0=mybir.AluOpType.add, op1=mybir.AluOpType.add)

        def finalize(l):
            rec = sb.tile([P, NBLK, 4], F32, tag="rec")
            nc.vector.reciprocal(out=rec[:], in_=acc[:, :, 128:132])
            nc.vector.tensor_tensor(
                out=yv[:].rearrange("p s (c h) -> p s c h", c=32),
                in0=acc[:, :, 0:128].rearrange("p s (c h) -> p s c h", c=32),
                in1=rec[:].unsqueeze(2).to_broadcast([P, NBLK, 32, 4]),
                op=mybir.AluOpType.mult)
            if l < 2:
                nc.vector.tensor_tensor(
                    out=yv[:], in0=yv[:],
                    in1=b_sb[l][:].unsqueeze(1).to_broadcast([P, NBLK, P]),
                    op=mybir.AluOpType.add)
                elu_inplace(yv, NBLK, P)

        for _rep in range(repeat):
          for l in range(3):
            if l == 0:
                w_transform_x()
            else:
                w_transform_y(l)
            build_tables(l)
            run_edges()
            finalize(l)

        pv = sb.tile([P, NBLK], F32, tag="pv")
        for q0 in range(0, NBLK, 16):
            qn = min(16, NBLK - q0)
            h3 = sb.tile([P, 16, HID], F32, tag="h3")
            nc.vector.tensor_reduce(
                out=h3[:, :qn, :],
                in_=yv[:, q0:q0 + qn, :].rearrange("p s (c h) -> p s c h", c=32),
                axis=mybir.AxisListType.X, op=mybir.AluOpType.add)
            nc.vector.tensor_scalar(out=h3[:, :qn, :], in0=h3[:, :qn, :],
                                    scalar1=0.25, scalar2=None,
                                    op0=mybir.AluOpType.mult)
            nc.vector.tensor_tensor(
                out=h3[:, :qn, :], in0=h3[:, :qn, :],
                in1=b3_sb[:].unsqueeze(1).to_broadcast([P, qn, HID]),
                op=mybir.AluOpType.add)
            elu_inplace(h3[:, :qn, :], qn, HID)
            tmp3 = sb.tile([P, 16, HID], F32, tag="tmp3")
            nc.vector.tensor_tensor(
                out=tmp3[:, :qn, :], in0=h3[:, :qn, :],
                in1=lw_sb[:].unsqueeze(1).to_broadcast([P, qn, HID]),
                op=mybir.AluOpType.mult)
            nc.vector.tensor_reduce(out=pv[:, q0:q0 + qn], in_=tmp3[:, :qn, :],
                                    axis=mybir.AxisListType.X,
                                    op=mybir.AluOpType.add)
        pool_ps = ps.tile([64, 1], F32, tag="big", space="PSUM")
        for s in range(NBLK):
            bps = sb.tile([P, 64], F32, tag="bps")
            nc.sync.dma_start(bps[:], bpool[:, s * 64:(s + 1) * 64])
            nc.tensor.matmul(pool_ps[:], lhsT=bps[:], rhs=pv[:, s:s + 1],
                             start=(s == 0), stop=(s == NBLK - 1))
        pool_sb = sb.tile([64, 1], F32, tag="poolsb")
        nc.scalar.copy(pool_sb[:], pool_ps[:])
        nc.sync.dma_start(out64[:], pool_sb[:])

    nc.compile()
    return nc


# ----------------------------------------------------------------------------
# host-side input construction
# ----------------------------------------------------------------------------
def make_inputs(sched, idx_arrs, mask_arrs, inputs, batch_counts=None):
    """Per-core in_maps from the raw problem inputs dict.

    Features are re-laid-out c-major (f_new = c*4 + h) so the per-head
    attention broadcast lands on a packed last dim (DVE 2x mode).
    """
    SH, NBLK, TROW = sched["SH"], sched["NBLK"], sched["TROW"]
    NROW = NBLK * 128
    x = np.asarray(inputs["x"], np.float32)
    N = x.shape[0]
    batch = np.asarray(inputs["batch"], np.int64)
    NGr = 64 if batch_counts is None else len(batch_counts)
    counts = np.bincount(batch, minlength=NGr).astype(np.float32)
    counts[counts == 0] = 1.0

    pf = np.arange(P)
    perm = (pf % HEADS) * HID + pf // HEADS   # old h-major idx at new position

    def rep(v, width=P):
        v = np.asarray(v, np.float32).reshape(1, -1)
        return np.tile(v, (P, 1))

    W1 = np.asarray(inputs["W1"], np.float32)[perm, :]
    W2 = np.asarray(inputs["W2"], np.float32)[perm][:, perm]
    W3 = np.asarray(inputs["W3"], np.float32)[perm][:, perm]
    Ws = [np.ascontiguousarray(W.T) for W in (W1, W2, W3)]
    asr = [rep(np.asarray(inputs[k], np.float32).reshape(-1)[perm]).astype(
        np.float16) for k in ("a1s", "a2s", "a3s")]
    adr = [rep(np.asarray(inputs[k], np.float32).reshape(-1)[perm]).astype(
        np.float16) for k in ("a1d", "a2d", "a3d")]
    br = [rep(np.asarray(inputs["b1"], np.float32)[perm]),
          rep(np.asarray(inputs["b2"], np.float32)[perm])]
    b3r = rep(inputs["b3"])
    lwr = rep(np.asarray(inputs["lin_w"], np.float32).reshape(-1))
    idf32 = np.eye(P, dtype=np.float32)

    in_maps = []
    for c in range(NCORES):
        xs = np.zeros((TROW, P), np.float32)
        xs[0:SH] = x[c * SH:(c + 1) * SH]
        bp = np.zeros((NROW, 64), np.float32)
        b_loc = batch[c * SH:(c + 1) * SH]
        bp[np.arange(SH), b_loc] = 1.0 / counts[b_loc]
        m = {"xT": np.ascontiguousarray(xs.T), "idx16": idx_arrs[c],
             "mdram": mask_arrs[c][0], "mTdram": mask_arrs[c][1],
             "b3rep": b3r, "lwrep": lwr,
             "ident32": idf32,
             "bpool": np.ascontiguousarray(
                 bp.reshape(NBLK, P, 64).transpose(1, 0, 2).reshape(P, NBLK * 64))}
        for l in range(3):
            m[f"WT{l}"] = Ws[l]
            m[f"asrep{l}"] = asr[l]
            m[f"adrep{l}"] = adr[l]
        for l in range(2):
            m[f"brep{l}"] = br[l]
        in_maps.append(m)
    return in_maps


# ----------------------------------------------------------------------------
# SPMD runner (modeled on bass2jax.run_bass_via_pjrt, with reusable executable)
# ----------------------------------------------------------------------------
def make_runner(nc, in_maps):
    import jax
    import jax.numpy as jnp
    from jax.sharding import Mesh, PartitionSpec
    from jax.experimental.shard_map import shard_map
    from concourse import bass2jax, mybir as mb

    bass2jax.install_neuronx_cc_hook()
    n_cores = len(in_maps)
    part_name = nc.partition_id_tensor.name if nc.partition_id_tensor else None
    in_names, out_names, out_avals, zero_outs = [], [], [], []
    for alloc in nc.m.functions[0].allocations:
        if not isinstance(alloc, mb.MemoryLocationSet):
            continue
        name = alloc.memorylocations[0].name
        if alloc.kind == "ExternalInput":
            if name != part_name:
                in_names.append(name)
        elif alloc.kind == "ExternalOutput":
            out_names.append(name)
            shape = tuple(alloc.tensor_shape)
            dtype = mb.dt.np(alloc.dtype)
            out_avals.append(jax.core.ShapedArray(shape, dtype))
            zero_outs.append(np.zeros(shape, dtype))
    n_params = len(in_names)
    all_names = in_names + out_names
    if part_name is not None:
        all_names = all_names + [part_name]

    def _body(*args):
        operands = list(args)
        if part_name is not None:
            operands.append(bass2jax.partition_id_tensor())
        outs = bass2jax._bass_exec_p.bind(
            *operands, out_avals=tuple(out_avals), in_names=tuple(all_names),
            out_names=tuple(out_names), lowering_input_output_aliases=(),
            sim_require_finite=False, sim_require_nnan=False, nc=nc)
        return tuple(outs)

    devices = jax.devices()[:n_cores]
    mesh = Mesh(np.asarray(devices), ("core",))
    in_specs = (PartitionSpec("core"),) * (n_params + len(out_names))
    out_specs = (PartitionSpec("core"),) * len(out_names)
    fn = jax.jit(shard_map(_body, mesh=mesh, in_specs=in_specs,
                           out_specs=out_specs, check_rep=False))
    concat_in = [np.concatenate([np.asarray(in_maps[c][nm])
                                 for c in range(n_cores)], axis=0)
                 for nm in in_names]
    concat_zeros = [np.zeros((n_cores * z.shape[0], *z.shape[1:]), z.dtype)
                    for z in zero_outs]
    dev_in = [jax.device_put(
        a, jax.sharding.NamedSharding(mesh, PartitionSpec("core")))
        for a in concat_in + concat_zeros]

    def run():
        outs = fn(*dev_in)
        outs = [np.asarray(o) for o in outs]
        return [
            {nm: outs[i].reshape(n_cores, *out_avals[i].shape)[c]
             for i, nm in enumerate(out_names)}
            for c in range(n_cores)]
    return run


def kernel(**inputs):
    """Full-input distributed GAT kernel; returns pooled [64] float32."""
    inputs = {k: np.asarray(v) for k, v in inputs.items()}
    N = inputs["x"].shape[0]
    sched, idx_arrs, d8_arrs = preprocess(inputs["edge_index"], N)
    nc = build_program(sched)
    in_maps = make_inputs(sched, idx_arrs, d8_arrs, inputs)
    run = make_runner(nc, in_maps)
    kernel.last_runner = run          # exposed for test.py timing
    kernel.last_inputs = inputs
    results = run()
    partial = sum(r["out64"][:, 0] for r in results)
    out = (partial + np.float32(inputs["lin_b"].reshape(-1)[0]))[:64]
    return out.astype(np.float32)
